# revision 6
# baseline (speedup 1.0000x reference)
"""Cross-attention Trainium2 kernel (8 NeuronCores, batch-parallel).

Reference computation (per batch element b):
    q = x @ Wq.T ; k = y @ Wk.T ; v = y @ Wv.T          (heads = 8, head_dim = 96)
    S = q k^T * scale + relative_pos                     ([h, n, m])
    out = softmax(S, -1) @ v ; out = out @ Wp.T + bp

Strategy:
  - one batch element per NeuronCore (B == 8 == n_cores), no collectives
  - host-side (free) pre-transposes: xT/yT [C, N]; WqT/WkT/WvT [C, C];
    Wp head-major [HD, H, C]; rel transposed [H, m, n] in fp16
  - on-device: Q.T/K.T head-major [HD, H, N]; V with appended ones column;
    scores computed transposed (S.T[m, n]) so the attention*V matmul needs no
    transposes; the ones column makes the softmax denominator fall out of the
    same matmul (row 96 of out_aug); softmax skips max-subtraction (|S| < ~7,
    exp is safe in fp32)
  - all matmuls in float32r (full PE rate, ~FP22 mantissa)
"""

import numpy as np
from contextlib import ExitStack

import concourse.bass as bass
import concourse.mybir as mybir
import concourse.tile as tile
from concourse import bacc
from concourse.bass_utils import run_bass_kernel_spmd

B, N, C = 8, 1024, 768
H, HD = 8, 96
KCH = C // 128     # 6 contraction chunks
NCH = N // 128     # 8 sequence chunks
SCALE = HD ** -0.5
F32 = mybir.dt.float32
F16 = mybir.dt.float16
F32R = mybir.dt.float32r
ADD = mybir.AluOpType.add
MUL = mybir.AluOpType.mult
EXP = mybir.ActivationFunctionType.Exp
LN = mybir.ActivationFunctionType.Ln

_CACHE = {}


def build_bass():
    if "nc" in _CACHE:
        return _CACHE["nc"]
    nc = bacc.Bacc("TRN2", target_bir_lowering=False, debug=False, num_devices=B)

    xT = nc.dram_tensor("xT", [C, N], F32R, kind="ExternalInput").ap()
    yT = nc.dram_tensor("yT", [C, N], F32R, kind="ExternalInput").ap()
    wq = nc.dram_tensor("wq", [C, C], F32R, kind="ExternalInput").ap()
    wk = nc.dram_tensor("wk", [C, C], F32R, kind="ExternalInput").ap()
    wv = nc.dram_tensor("wv", [C, C], F32R, kind="ExternalInput").ap()
    wp = nc.dram_tensor("wp", [HD, H, C], F32R, kind="ExternalInput").ap()
    bp = nc.dram_tensor("bp", [1, C], F32R, kind="ExternalInput").ap()
    rel = nc.dram_tensor("rel", [H, N, N], F16, kind="ExternalInput").ap()
    onesr = nc.dram_tensor("onesr", [1, 128], F32R, kind="ExternalInput").ap()
    onesv = nc.dram_tensor("onesv", [128, NCH * H], F32R, kind="ExternalInput").ap()
    out = nc.dram_tensor("out", [N, C], F32, kind="ExternalOutput").ap()

    with tile.TileContext(nc) as tc:
        with ExitStack() as ctx:
            ps = ctx.enter_context(tc.tile_pool(name="ps", bufs=8, space="PSUM"))
            qk_pool = ctx.enter_context(tc.tile_pool(name="qk", bufs=2))
            vaug_pool = ctx.enter_context(tc.tile_pool(name="vaug", bufs=1))
            at_pool = ctx.enter_context(tc.tile_pool(name="at", bufs=1))
            const_pool = ctx.enter_context(tc.tile_pool(name="const", bufs=1))

            ones = const_pool.tile([1, 128], F32R)
            nc.sync.dma_start(ones[:], onesr[:])
            bp_sb = const_pool.tile([1, C], F32R)
            nc.sync.dma_start(bp_sb[:], bp[:])

            qth = qk_pool.tile([HD, H, N], F32R, tag="qk", name="qth")
            kth = qk_pool.tile([HD, H, N], F32R, tag="qk", name="kth")
            # V, head-padded, with a ones column at index HD (DMA'd from host;
            # walrus rejects Memset on fp32r)
            vaug = vaug_pool.tile([128, NCH, H, HD + 1], F32R)
            nc.sync.dma_start(vaug[:, :, :, HD], onesv.rearrange("p (a b) -> p a b", a=NCH))
            at_hm = at_pool.tile([HD, H, N], F32R)  # normalized attn-out, head-major

            # ---------------- Q.T / K.T / V projections ----------------
            with ExitStack() as qkv_ctx:
                w_pool = qkv_ctx.enter_context(tc.tile_pool(name="w", bufs=2))
                x_pool = qkv_ctx.enter_context(tc.tile_pool(name="x", bufs=2))
                y_pool = qkv_ctx.enter_context(tc.tile_pool(name="y", bufs=1))

                yt_sb = y_pool.tile([128, KCH, N], F32R)
                nc.sync.dma_start(yt_sb[:], yT.rearrange("(ko ki) n -> ki ko n", ki=128))

                # Q.T and K.T, head-major [HD, H, N]
                for which, w_dram, dst, scale in ((0, wq, qth, SCALE), (1, wk, kth, 1.0)):
                    for nb in range(2):
                        pst = [
                            ps.tile([128, 512], F32, tag="ps", name=f"qk{which}_{nb}_{h}")
                            for h in range(H)
                        ]
                        for k in range(KCH):
                            w_t = w_pool.tile([128, C], F32R, tag="w")
                            nc.sync.dma_start(w_t[:], w_dram[k * 128:(k + 1) * 128, :])
                            if which == 0:
                                rhs_t = x_pool.tile([128, 512], F32R, tag="x")
                                nc.sync.dma_start(
                                    rhs_t[:], xT[k * 128:(k + 1) * 128, nb * 512:(nb + 1) * 512]
                                )
                                rhs = rhs_t[:]
                            else:
                                rhs = yt_sb[:, k, nb * 512:(nb + 1) * 512]
                            for h in range(H):
                                nc.tensor.matmul(
                                    pst[h][:HD, :],
                                    w_t[:, h * HD:(h + 1) * HD],
                                    rhs,
                                    start=(k == 0),
                                    stop=(k == KCH - 1),
                                )
                        for h in range(H):
                            nc.scalar.mul(dst[:, h, nb * 512:(nb + 1) * 512], pst[h][:HD, :], scale)

                # V (natural [m, c] layout scattered into vaug head slots)
                for c0, cw in ((0, 512), (512, 256)):
                    psv = [
                        ps.tile([128, 512], F32, tag="ps", name=f"v{c0}_{mc}")
                        for mc in range(NCH)
                    ]
                    for k in range(KCH):
                        w_t = w_pool.tile([128, C], F32R, tag="w")
                        nc.sync.dma_start(w_t[:, :cw], wv[k * 128:(k + 1) * 128, c0:c0 + cw])
                        for mc in range(NCH):
                            nc.tensor.matmul(
                                psv[mc][:, :cw],
                                yt_sb[:, k, mc * 128:(mc + 1) * 128],
                                w_t[:, :cw],
                                start=(k == 0),
                                stop=(k == KCH - 1),
                            )
                    for mc in range(NCH):
                        c = c0
                        i = 0
                        while c < c0 + cw:
                            h = c // HD
                            hi = min((h + 1) * HD, c0 + cw)
                            dst_ap = vaug[:, mc, h, c - h * HD:hi - h * HD]
                            src_ap = psv[mc][:, c - c0:hi - c0]
                            if i % 2 == 0:
                                nc.vector.tensor_copy(dst_ap, src_ap)
                            else:
                                nc.scalar.copy(dst_ap, src_ap)
                            c = hi
                            i += 1

            # ---------------- attention ----------------
            with ExitStack() as att_ctx:
                rel_pool = att_ctx.enter_context(tc.tile_pool(name="rel", bufs=2))
                es_pool = att_ctx.enter_context(tc.tile_pool(name="es", bufs=3))
                bc_pool = att_ctx.enter_context(tc.tile_pool(name="bc", bufs=3))
                sm_pool = att_ctx.enter_context(tc.tile_pool(name="sm", bufs=2))

                for h in range(H):
                    oa0 = ps.tile([128, 512], F32, tag="ps", name=f"oa0_{h}")
                    oa1 = ps.tile([128, 512], F32, tag="ps", name=f"oa1_{h}")
                    for mc in range(NCH):
                        st0 = ps.tile([128, 512], F32, tag="ps", name="st0")
                        st1 = ps.tile([128, 512], F32, tag="ps", name="st1")
                        kt_sl = kth[:, h, mc * 128:(mc + 1) * 128]
                        nc.tensor.matmul(st0[:], kt_sl, qth[:, h, 0:512], start=True, stop=True)
                        nc.tensor.matmul(st1[:], kt_sl, qth[:, h, 512:1024], start=True, stop=True)
                        rel_t = rel_pool.tile([128, N], F16, tag="rel")
                        nc.sync.dma_start(rel_t[:], rel[h, mc * 128:(mc + 1) * 128, :])
                        es = es_pool.tile([128, N], F32R, tag="es")
                        nc.vector.tensor_tensor(es[:, 0:512], st0[:], rel_t[:, 0:512], ADD)
                        nc.vector.tensor_tensor(es[:, 512:1024], st1[:], rel_t[:, 512:1024], ADD)
                        nc.scalar.activation(es[:], es[:], EXP)
                        va = vaug[:, mc, h, :]
                        nc.tensor.matmul(oa0[:HD + 1, :], va, es[:, 0:512],
                                         start=(mc == 0), stop=(mc == NCH - 1))
                        nc.tensor.matmul(oa1[:HD + 1, :], va, es[:, 512:1024],
                                         start=(mc == 0), stop=(mc == NCH - 1))
                    # normalize: row HD of oa* holds the softmax denominators per n
                    sums = sm_pool.tile([128, N], F32R, tag="sm")
                    nc.scalar.copy(sums[HD:HD + 1, 0:512], oa0[HD:HD + 1, :])
                    nc.scalar.copy(sums[HD:HD + 1, 512:1024], oa1[HD:HD + 1, :])
                    # partition shift 96 -> 0 (DMA is the only engine that can)
                    nc.sync.dma_start(sums[0:1, :], sums[HD:HD + 1, :])
                    bc0 = ps.tile([128, 512], F32, tag="ps", name="bc0")
                    bc1 = ps.tile([128, 512], F32, tag="ps", name="bc1")
                    nc.tensor.matmul(bc0[:HD, :], ones[0:1, :HD], sums[0:1, 0:512],
                                     start=True, stop=True)
                    nc.tensor.matmul(bc1[:HD, :], ones[0:1, :HD], sums[0:1, 512:1024],
                                     start=True, stop=True)
                    # 1/x as exp(-ln(x)) on ACT (DVE reciprocal is 8 cyc/elem)
                    t1 = bc_pool.tile([HD, N], F32, tag="bc")
                    nc.scalar.activation(t1[:, 0:512], bc0[:HD, :], LN)
                    nc.scalar.activation(t1[:, 512:1024], bc1[:HD, :], LN)
                    rcp = bc_pool.tile([HD, N], F32, tag="bc")
                    nc.scalar.activation(rcp[:], t1[:], EXP, scale=-1.0)
                    nc.vector.tensor_tensor(at_hm[:, h, 0:512], oa0[:HD, :], rcp[:, 0:512], MUL)
                    nc.vector.tensor_tensor(at_hm[:, h, 512:1024], oa1[:HD, :], rcp[:, 512:1024], MUL)

            # ---------------- output projection ----------------
            with ExitStack() as proj_ctx:
                wp_pool = proj_ctx.enter_context(tc.tile_pool(name="wpp", bufs=2))
                ob_pool = proj_ctx.enter_context(tc.tile_pool(name="ob", bufs=3))
                for c0, cw in ((0, 512), (512, 256)):
                    po = [
                        ps.tile([128, 512], F32, tag="ps", name=f"po{c0}_{j}")
                        for j in range(NCH)
                    ]
                    for h in range(H):
                        wp_t = wp_pool.tile([HD, 512], F32R, tag="wpp")
                        nc.sync.dma_start(wp_t[:, :cw], wp[:, h, c0:c0 + cw])
                        for j in range(NCH):
                            nc.tensor.matmul(
                                po[j][:, :cw],
                                at_hm[:, h, j * 128:(j + 1) * 128],
                                wp_t[:, :cw],
                                start=(h == 0),
                                stop=False,
                            )
                    for j in range(NCH):
                        # + bias via K=1 ones matmul (broadcast bp over partitions)
                        nc.tensor.matmul(po[j][:, :cw], ones[0:1, :], bp_sb[0:1, c0:c0 + cw],
                                         start=False, stop=True)
                        ot = ob_pool.tile([128, 512], F32, tag="ob")
                        nc.vector.tensor_copy(ot[:, :cw], po[j][:, :cw])
                        nc.sync.dma_start(out[j * 128:(j + 1) * 128, c0:c0 + cw], ot[:, :cw])

    nc.compile()
    _CACHE["nc"] = nc
    return nc


def make_in_maps(x, y, relative_pos, Wq, Wk, Wv, Wp, bp):
    x = np.asarray(x, dtype=np.float32)
    y = np.asarray(y, dtype=np.float32)
    relative_pos = np.asarray(relative_pos, dtype=np.float32)
    Wq = np.asarray(Wq, dtype=np.float32)
    Wk = np.asarray(Wk, dtype=np.float32)
    Wv = np.asarray(Wv, dtype=np.float32)
    Wp = np.asarray(Wp, dtype=np.float32)
    bp = np.asarray(bp, dtype=np.float32)

    wqT = np.ascontiguousarray(Wq.T)
    wkT = np.ascontiguousarray(Wk.T)
    wvT = np.ascontiguousarray(Wv.T)
    # Wp.T is [c'=h*HD+d, c]; head-major: [d, h, c]
    wp_hm = np.ascontiguousarray(Wp.T.reshape(H, HD, C).transpose(1, 0, 2))
    relT = np.ascontiguousarray(relative_pos.transpose(0, 2, 1)).astype(np.float16)
    bp2 = np.ascontiguousarray(bp.reshape(1, C))

    in_maps = []
    for b in range(B):
        in_maps.append({
            "xT": np.ascontiguousarray(x[b].T),
            "yT": np.ascontiguousarray(y[b].T),
            "wq": wqT, "wk": wkT, "wv": wvT, "wp": wp_hm, "bp": bp2,
            "rel": relT,
            "onesr": np.ones((1, 128), dtype=np.float32),
            "onesv": np.ones((128, NCH * H), dtype=np.float32),
        })
    return in_maps


def kernel(x, y, relative_pos, H=None, W=None, Wq=None, Wk=None, Wv=None, Wp=None, bp=None,
           **extra):
    nc = build_bass()
    in_maps = make_in_maps(x, y, relative_pos, Wq, Wk, Wv, Wp, bp)
    res = run_bass_kernel_spmd(nc, in_maps, list(range(B)))
    return np.stack([res.results[b]["out"] for b in range(B)], axis=0)


# revision 7
# speedup vs baseline: 1.1288x; 1.1288x over previous
"""Cross-attention Trainium2 kernel (8 NeuronCores, batch-parallel).

Reference computation (per batch element b):
    q = x @ Wq.T ; k = y @ Wk.T ; v = y @ Wv.T          (heads = 8, head_dim = 96)
    S = q k^T * scale + relative_pos                     ([h, n, m])
    out = softmax(S, -1) @ v ; out = out @ Wp.T + bp

Strategy:
  - one batch element per NeuronCore (B == 8 == n_cores), no collectives
  - host-side (free) pre-transposes: xT/yT [C, N]; WqT/WkT/WvT [C, C];
    Wp head-major [HD, H, C]; rel transposed [H, m, n] in fp16
  - on-device: Q.T/K.T head-major [HD, H, N]; V with appended ones column;
    scores computed transposed (S.T[m, n]) so the attention*V matmul needs no
    transposes; the ones column makes the softmax denominator fall out of the
    same matmul (row 96 of out_aug); softmax skips max-subtraction (|S| < ~7,
    exp is safe in fp32)
  - all matmuls in float32r (full PE rate, ~FP22 mantissa)
"""

import numpy as np
from contextlib import ExitStack

import concourse.bass as bass
import concourse.mybir as mybir
import concourse.tile as tile
from concourse import bacc
from concourse.bass_utils import run_bass_kernel_spmd

B, N, C = 8, 1024, 768
H, HD = 8, 96
KCH = C // 128     # 6 contraction chunks
NCH = N // 128     # 8 sequence chunks
SCALE = HD ** -0.5
F32 = mybir.dt.float32
F16 = mybir.dt.float16
F32R = mybir.dt.float32r
ADD = mybir.AluOpType.add
MUL = mybir.AluOpType.mult
EXP = mybir.ActivationFunctionType.Exp
LN = mybir.ActivationFunctionType.Ln

_CACHE = {}


def build_bass():
    if "nc" in _CACHE:
        return _CACHE["nc"]
    nc = bacc.Bacc("TRN2", target_bir_lowering=False, debug=False, num_devices=B)

    xT = nc.dram_tensor("xT", [C, N], F32R, kind="ExternalInput").ap()
    yT = nc.dram_tensor("yT", [C, N], F32R, kind="ExternalInput").ap()
    wq = nc.dram_tensor("wq", [C, C], F32R, kind="ExternalInput").ap()
    wk = nc.dram_tensor("wk", [C, C], F32R, kind="ExternalInput").ap()
    wv = nc.dram_tensor("wv", [C, C], F32R, kind="ExternalInput").ap()
    wp = nc.dram_tensor("wp", [HD, H, C], F32R, kind="ExternalInput").ap()
    bp = nc.dram_tensor("bp", [1, C], F32R, kind="ExternalInput").ap()
    rel = nc.dram_tensor("rel", [H, N, N], F16, kind="ExternalInput").ap()
    onesr = nc.dram_tensor("onesr", [1, 128], F32R, kind="ExternalInput").ap()
    ident = nc.dram_tensor("ident", [128, 128], F16, kind="ExternalInput").ap()
    onesv = nc.dram_tensor("onesv", [128, NCH * H], F32R, kind="ExternalInput").ap()
    out = nc.dram_tensor("out", [N, C], F32, kind="ExternalOutput").ap()

    with tile.TileContext(nc) as tc:
        with ExitStack() as ctx:
            ps = ctx.enter_context(tc.tile_pool(name="ps", bufs=8, space="PSUM"))
            qk_pool = ctx.enter_context(tc.tile_pool(name="qk", bufs=2))
            vaug_pool = ctx.enter_context(tc.tile_pool(name="vaug", bufs=1))
            at_pool = ctx.enter_context(tc.tile_pool(name="at", bufs=1))
            const_pool = ctx.enter_context(tc.tile_pool(name="const", bufs=1))

            ones = const_pool.tile([1, 128], F32R)
            nc.sync.dma_start(ones[:], onesr[:])
            bp_sb = const_pool.tile([1, C], F32R)
            nc.sync.dma_start(bp_sb[:], bp[:])
            id_sb = const_pool.tile([128, 128], F16)
            nc.sync.dma_start(id_sb[:], ident[:])

            qth = qk_pool.tile([HD, H, N], F32R, tag="qk", name="qth")
            kth = qk_pool.tile([HD, H, N], F32R, tag="qk", name="kth")
            # V, head-padded, with a ones column at index HD (DMA'd from host;
            # walrus rejects Memset on fp32r)
            vaug = vaug_pool.tile([128, NCH, H, HD + 1], F32R)
            nc.gpsimd.dma_start(vaug[:, :, :, HD], onesv.rearrange("p (a b) -> p a b", a=NCH))
            at_hm = at_pool.tile([HD, H, N], F32R)  # normalized attn-out, head-major

            # ---------------- Q.T / K.T / V projections ----------------
            with ExitStack() as qkv_ctx:
                w_pool = qkv_ctx.enter_context(tc.tile_pool(name="w", bufs=2))
                x_pool = qkv_ctx.enter_context(tc.tile_pool(name="x", bufs=2))
                y_pool = qkv_ctx.enter_context(tc.tile_pool(name="y", bufs=1))

                yt_sb = y_pool.tile([128, KCH, N], F32R)
                nc.gpsimd.dma_start(yt_sb[:], yT.rearrange("(ko ki) n -> ki ko n", ki=128))

                # Q.T and K.T, head-major [HD, H, N]
                for which, w_dram, dst, scale in ((0, wq, qth, SCALE), (1, wk, kth, 1.0)):
                    for nb in range(2):
                        pst = [
                            ps.tile([128, 512], F32, tag="ps", name=f"qk{which}_{nb}_{h}")
                            for h in range(H)
                        ]
                        for k in range(KCH):
                            w_t = w_pool.tile([128, C], F32R, tag="w")
                            nc.sync.dma_start(w_t[:], w_dram[k * 128:(k + 1) * 128, :])
                            if which == 0:
                                rhs_t = x_pool.tile([128, 512], F32R, tag="x")
                                nc.sync.dma_start(
                                    rhs_t[:], xT[k * 128:(k + 1) * 128, nb * 512:(nb + 1) * 512]
                                )
                                rhs = rhs_t[:]
                            else:
                                rhs = yt_sb[:, k, nb * 512:(nb + 1) * 512]
                            for h in range(H):
                                nc.tensor.matmul(
                                    pst[h][:HD, :],
                                    w_t[:, h * HD:(h + 1) * HD],
                                    rhs,
                                    start=(k == 0),
                                    stop=(k == KCH - 1),
                                )
                        for h in range(H):
                            if which == 0:
                                nc.scalar.mul(dst[:, h, nb * 512:(nb + 1) * 512], pst[h][:HD, :], scale)
                            else:
                                nc.vector.tensor_copy(dst[:, h, nb * 512:(nb + 1) * 512], pst[h][:HD, :])

                # V (natural [m, c] layout scattered into vaug head slots)
                for c0, cw in ((0, 512), (512, 256)):
                    psv = [
                        ps.tile([128, 512], F32, tag="ps", name=f"v{c0}_{mc}")
                        for mc in range(NCH)
                    ]
                    for k in range(KCH):
                        w_t = w_pool.tile([128, C], F32R, tag="w")
                        nc.sync.dma_start(w_t[:, :cw], wv[k * 128:(k + 1) * 128, c0:c0 + cw])
                        for mc in range(NCH):
                            nc.tensor.matmul(
                                psv[mc][:, :cw],
                                yt_sb[:, k, mc * 128:(mc + 1) * 128],
                                w_t[:, :cw],
                                start=(k == 0),
                                stop=(k == KCH - 1),
                            )
                    for mc in range(NCH):
                        c = c0
                        i = 0
                        while c < c0 + cw:
                            h = c // HD
                            hi = min((h + 1) * HD, c0 + cw)
                            dst_ap = vaug[:, mc, h, c - h * HD:hi - h * HD]
                            src_ap = psv[mc][:, c - c0:hi - c0]
                            if i % 2 == 0:
                                nc.vector.tensor_copy(dst_ap, src_ap)
                            else:
                                nc.scalar.copy(dst_ap, src_ap)
                            c = hi
                            i += 1

            # ---------------- attention ----------------
            with ExitStack() as att_ctx:
                rel_pool = att_ctx.enter_context(tc.tile_pool(name="rel", bufs=3))
                es_pool = att_ctx.enter_context(tc.tile_pool(name="es", bufs=4))
                bc_pool = att_ctx.enter_context(tc.tile_pool(name="bc", bufs=3))
                sm_pool = att_ctx.enter_context(tc.tile_pool(name="sm", bufs=2))

                for h in range(H):
                    oa0 = ps.tile([128, 512], F32, tag="ps", name=f"oa0_{h}")
                    oa1 = ps.tile([128, 512], F32, tag="ps", name=f"oa1_{h}")
                    for mc in range(NCH):
                        st0 = ps.tile([128, 512], F32, tag="ps", name="st0")
                        st1 = ps.tile([128, 512], F32, tag="ps", name="st1")
                        kt_sl = kth[:, h, mc * 128:(mc + 1) * 128]
                        rel_t = rel_pool.tile([128, N], F16, tag="rel")
                        nc.gpsimd.dma_start(rel_t[:], rel[h, mc * 128:(mc + 1) * 128, :])
                        nc.tensor.matmul(st0[:], kt_sl, qth[:, h, 0:512], start=True, stop=False)
                        nc.tensor.matmul(st0[:], id_sb[:], rel_t[:, 0:512], start=False, stop=True)
                        nc.tensor.matmul(st1[:], kt_sl, qth[:, h, 512:1024], start=True, stop=False)
                        nc.tensor.matmul(st1[:], id_sb[:], rel_t[:, 512:1024], start=False, stop=True)
                        es = es_pool.tile([128, N], F32R, tag="es")
                        nc.scalar.activation(es[:, 0:512], st0[:], EXP)
                        nc.scalar.activation(es[:, 512:1024], st1[:], EXP)
                        va = vaug[:, mc, h, :]
                        nc.tensor.matmul(oa0[:HD + 1, :], va, es[:, 0:512],
                                         start=(mc == 0), stop=(mc == NCH - 1))
                        nc.tensor.matmul(oa1[:HD + 1, :], va, es[:, 512:1024],
                                         start=(mc == 0), stop=(mc == NCH - 1))
                    # normalize: row HD of oa* holds the softmax denominators per n
                    sums = sm_pool.tile([128, N], F32R, tag="sm")
                    nc.scalar.copy(sums[HD:HD + 1, 0:512], oa0[HD:HD + 1, :])
                    nc.scalar.copy(sums[HD:HD + 1, 512:1024], oa1[HD:HD + 1, :])
                    # partition shift 96 -> 0 (DMA is the only engine that can)
                    nc.sync.dma_start(sums[0:1, :], sums[HD:HD + 1, :])
                    bc0 = ps.tile([128, 512], F32, tag="ps", name="bc0")
                    bc1 = ps.tile([128, 512], F32, tag="ps", name="bc1")
                    nc.tensor.matmul(bc0[:HD, :], ones[0:1, :HD], sums[0:1, 0:512],
                                     start=True, stop=True)
                    nc.tensor.matmul(bc1[:HD, :], ones[0:1, :HD], sums[0:1, 512:1024],
                                     start=True, stop=True)
                    # 1/x via DVE Newton-Raphson custom op (~2 ULP)
                    scr = bc_pool.tile([HD, N], F32, tag="bc")
                    rcp = bc_pool.tile([HD, N], F32, tag="bc")
                    nc.vector.reciprocal_approx_accurate(rcp[:, 0:512], bc0[:HD, :], scr[:, 0:512])
                    nc.vector.reciprocal_approx_accurate(rcp[:, 512:1024], bc1[:HD, :], scr[:, 512:1024])
                    nc.vector.tensor_tensor(at_hm[:, h, 0:512], oa0[:HD, :], rcp[:, 0:512], MUL)
                    nc.vector.tensor_tensor(at_hm[:, h, 512:1024], oa1[:HD, :], rcp[:, 512:1024], MUL)

            # ---------------- output projection ----------------
            with ExitStack() as proj_ctx:
                wp_pool = proj_ctx.enter_context(tc.tile_pool(name="wpp", bufs=2))
                ob_pool = proj_ctx.enter_context(tc.tile_pool(name="ob", bufs=3))
                for c0, cw in ((0, 512), (512, 256)):
                    po = [
                        ps.tile([128, 512], F32, tag="ps", name=f"po{c0}_{j}")
                        for j in range(NCH)
                    ]
                    for h in range(H):
                        wp_t = wp_pool.tile([HD, 512], F32R, tag="wpp")
                        nc.sync.dma_start(wp_t[:, :cw], wp[:, h, c0:c0 + cw])
                        for j in range(NCH):
                            nc.tensor.matmul(
                                po[j][:, :cw],
                                at_hm[:, h, j * 128:(j + 1) * 128],
                                wp_t[:, :cw],
                                start=(h == 0),
                                stop=False,
                            )
                    for j in range(NCH):
                        # + bias via K=1 ones matmul (broadcast bp over partitions)
                        nc.tensor.matmul(po[j][:, :cw], ones[0:1, :], bp_sb[0:1, c0:c0 + cw],
                                         start=False, stop=True)
                        ot = ob_pool.tile([128, 512], F32, tag="ob")
                        nc.vector.tensor_copy(ot[:, :cw], po[j][:, :cw])
                        nc.sync.dma_start(out[j * 128:(j + 1) * 128, c0:c0 + cw], ot[:, :cw])

    nc.compile()
    _CACHE["nc"] = nc
    return nc


def make_in_maps(x, y, relative_pos, Wq, Wk, Wv, Wp, bp):
    x = np.asarray(x, dtype=np.float32)
    y = np.asarray(y, dtype=np.float32)
    relative_pos = np.asarray(relative_pos, dtype=np.float32)
    Wq = np.asarray(Wq, dtype=np.float32)
    Wk = np.asarray(Wk, dtype=np.float32)
    Wv = np.asarray(Wv, dtype=np.float32)
    Wp = np.asarray(Wp, dtype=np.float32)
    bp = np.asarray(bp, dtype=np.float32)

    wqT = np.ascontiguousarray(Wq.T)
    wkT = np.ascontiguousarray(Wk.T)
    wvT = np.ascontiguousarray(Wv.T)
    # Wp.T is [c'=h*HD+d, c]; head-major: [d, h, c]
    wp_hm = np.ascontiguousarray(Wp.T.reshape(H, HD, C).transpose(1, 0, 2))
    relT = np.ascontiguousarray(relative_pos.transpose(0, 2, 1)).astype(np.float16)
    bp2 = np.ascontiguousarray(bp.reshape(1, C))

    in_maps = []
    for b in range(B):
        in_maps.append({
            "xT": np.ascontiguousarray(x[b].T),
            "yT": np.ascontiguousarray(y[b].T),
            "wq": wqT, "wk": wkT, "wv": wvT, "wp": wp_hm, "bp": bp2,
            "rel": relT,
            "onesr": np.ones((1, 128), dtype=np.float32),
            "ident": np.eye(128, dtype=np.float16),
            "onesv": np.ones((128, NCH * H), dtype=np.float32),
        })
    return in_maps


def kernel(x, y, relative_pos, H=None, W=None, Wq=None, Wk=None, Wv=None, Wp=None, bp=None,
           **extra):
    nc = build_bass()
    in_maps = make_in_maps(x, y, relative_pos, Wq, Wk, Wv, Wp, bp)
    res = run_bass_kernel_spmd(nc, in_maps, list(range(B)))
    return np.stack([res.results[b]["out"] for b in range(B)], axis=0)


# revision 9
# speedup vs baseline: 1.2598x; 1.1160x over previous
"""Cross-attention Trainium2 kernel (8 NeuronCores, batch-parallel).

Reference computation (per batch element b):
    q = x @ Wq.T ; k = y @ Wk.T ; v = y @ Wv.T          (heads = 8, head_dim = 96)
    S = q k^T * scale + relative_pos                     ([h, n, m])
    out = softmax(S, -1) @ v ; out = out @ Wp.T + bp

Strategy:
  - one batch element per NeuronCore (B == 8 == n_cores), no collectives
  - host-side (free) pre-transposes: xT/yT [C, N]; WqT/WkT/WvT [C, C];
    Wp head-major [HD, H, C]; rel transposed [H, m, n] in fp16
  - on-device: Q.T/K.T head-major [HD, H, N]; V with appended ones column;
    scores computed transposed (S.T[m, n]) so the attention*V matmul needs no
    transposes; the ones column makes the softmax denominator fall out of the
    same matmul (row 96 of out_aug); softmax skips max-subtraction (|S| < ~7,
    exp is safe in fp32)
  - all matmuls in float32r (full PE rate, ~FP22 mantissa)
"""

import numpy as np
from contextlib import ExitStack

import concourse.bass as bass
import concourse.mybir as mybir
import concourse.tile as tile
from concourse import bacc
from concourse.bass_utils import run_bass_kernel_spmd

B, N, C = 8, 1024, 768
H, HD = 8, 96
KCH = C // 128     # 6 contraction chunks
NCH = N // 128     # 8 sequence chunks
SCALE = HD ** -0.5
F32 = mybir.dt.float32
F16 = mybir.dt.float16
F32R = mybir.dt.float32r
ADD = mybir.AluOpType.add
MUL = mybir.AluOpType.mult
EXP = mybir.ActivationFunctionType.Exp
LN = mybir.ActivationFunctionType.Ln

_CACHE = {}


def build_bass():
    if "nc" in _CACHE:
        return _CACHE["nc"]
    nc = bacc.Bacc("TRN2", target_bir_lowering=False, debug=False, num_devices=B)

    xT = nc.dram_tensor("xT", [C, N], F32R, kind="ExternalInput").ap()
    yT = nc.dram_tensor("yT", [C, N], F32R, kind="ExternalInput").ap()
    wq = nc.dram_tensor("wq", [C, C], F32R, kind="ExternalInput").ap()
    wk = nc.dram_tensor("wk", [C, C], F32R, kind="ExternalInput").ap()
    wv = nc.dram_tensor("wv", [C, C], F32R, kind="ExternalInput").ap()
    wp = nc.dram_tensor("wp", [HD, H, C], F32R, kind="ExternalInput").ap()
    bp = nc.dram_tensor("bp", [1, C], F32R, kind="ExternalInput").ap()
    rel = nc.dram_tensor("rel", [H, N, N], F16, kind="ExternalInput").ap()
    onesr = nc.dram_tensor("onesr", [1, 128], F32R, kind="ExternalInput").ap()
    ident = nc.dram_tensor("ident", [128, 128], F16, kind="ExternalInput").ap()
    onesv = nc.dram_tensor("onesv", [128, NCH * H], F32R, kind="ExternalInput").ap()
    out = nc.dram_tensor("out", [N, C], F32, kind="ExternalOutput").ap()

    with tile.TileContext(nc) as tc:
        with ExitStack() as ctx:
            ps_a = ctx.enter_context(tc.tile_pool(name="ps_a", bufs=4, space="PSUM"))
            ps_b = ctx.enter_context(tc.tile_pool(name="ps_b", bufs=4, space="PSUM"))
            qk_pool = ctx.enter_context(tc.tile_pool(name="qk", bufs=2))
            vaug_pool = ctx.enter_context(tc.tile_pool(name="vaug", bufs=1))
            at_pool = ctx.enter_context(tc.tile_pool(name="at", bufs=1))
            const_pool = ctx.enter_context(tc.tile_pool(name="const", bufs=1))

            ones = const_pool.tile([1, 128], F32R)
            nc.sync.dma_start(ones[:], onesr[:])
            bp_sb = const_pool.tile([1, C], F32R)
            nc.sync.dma_start(bp_sb[:], bp[:])
            id_sb = const_pool.tile([128, 128], F16)
            nc.sync.dma_start(id_sb[:], ident[:])

            qth = qk_pool.tile([HD, H, N], F32R, tag="qk", name="qth")
            kth = qk_pool.tile([HD, H, N], F32R, tag="qk", name="kth")
            # V, head-padded, with a ones column at index HD (DMA'd from host;
            # walrus rejects Memset on fp32r)
            vaug = vaug_pool.tile([128, NCH, H, HD + 1], F32R)
            nc.gpsimd.dma_start(vaug[:, :, :, HD], onesv.rearrange("p (a b) -> p a b", a=NCH))
            at_hm = at_pool.tile([HD, H, N], F32R)  # normalized attn-out, head-major

            # ---------------- Q.T / K.T / V projections ----------------
            with ExitStack() as qkv_ctx:
                w_pool = qkv_ctx.enter_context(tc.tile_pool(name="w", bufs=3))
                x_pool = qkv_ctx.enter_context(tc.tile_pool(name="x", bufs=4))
                y_pool = qkv_ctx.enter_context(tc.tile_pool(name="y", bufs=1))

                yt_sb = y_pool.tile([128, KCH, N], F32R)
                nc.gpsimd.dma_start(yt_sb[:], yT.rearrange("(ko ki) n -> ki ko n", ki=128))

                # Q.T and K.T, head-major [HD, H, N]
                for which, w_dram, dst, scale in ((0, wq, qth, SCALE), (1, wk, kth, 1.0)):
                    for nb in range(2):
                        pst = [
                            (ps_a if h < 4 else ps_b).tile(
                                [128, 512], F32, tag="psa" if h < 4 else "psb",
                                name=f"qk{which}_{nb}_{h}")
                            for h in range(H)
                        ]
                        for k in range(KCH):
                            w_t = w_pool.tile([128, C], F32R, tag="w")
                            nc.sync.dma_start(w_t[:], w_dram[k * 128:(k + 1) * 128, :])
                            if which == 0:
                                rhs_t = x_pool.tile([128, 512], F32R, tag="x")
                                nc.scalar.dma_start(
                                    rhs_t[:], xT[k * 128:(k + 1) * 128, nb * 512:(nb + 1) * 512]
                                )
                                rhs = rhs_t[:]
                            else:
                                rhs = yt_sb[:, k, nb * 512:(nb + 1) * 512]
                            for h in range(H):
                                nc.tensor.matmul(
                                    pst[h][:HD, :],
                                    w_t[:, h * HD:(h + 1) * HD],
                                    rhs,
                                    start=(k == 0),
                                    stop=(k == KCH - 1),
                                )
                        for h in range(H):
                            if which == 0:
                                nc.scalar.mul(dst[:, h, nb * 512:(nb + 1) * 512], pst[h][:HD, :], scale)
                            else:
                                nc.vector.tensor_copy(dst[:, h, nb * 512:(nb + 1) * 512], pst[h][:HD, :])

                # V (natural [m, c] layout scattered into vaug head slots)
                for c0, cw in ((0, 512), (512, 256)):
                    psv = [
                        (ps_a if mc < 4 else ps_b).tile(
                            [128, 512], F32, tag="psa" if mc < 4 else "psb",
                            name=f"v{c0}_{mc}")
                        for mc in range(NCH)
                    ]
                    for k in range(KCH):
                        w_t = w_pool.tile([128, C], F32R, tag="w")
                        nc.sync.dma_start(w_t[:, :cw], wv[k * 128:(k + 1) * 128, c0:c0 + cw])
                        for mc in range(NCH):
                            nc.tensor.matmul(
                                psv[mc][:, :cw],
                                yt_sb[:, k, mc * 128:(mc + 1) * 128],
                                w_t[:, :cw],
                                start=(k == 0),
                                stop=(k == KCH - 1),
                            )
                    for mc in range(NCH):
                        c = c0
                        i = 0
                        while c < c0 + cw:
                            h = c // HD
                            hi = min((h + 1) * HD, c0 + cw)
                            dst_ap = vaug[:, mc, h, c - h * HD:hi - h * HD]
                            src_ap = psv[mc][:, c - c0:hi - c0]
                            if i % 2 == 0:
                                nc.vector.tensor_copy(dst_ap, src_ap)
                            else:
                                nc.scalar.copy(dst_ap, src_ap)
                            c = hi
                            i += 1

            # ---------------- attention ----------------
            with ExitStack() as att_ctx:
                rel_pool = att_ctx.enter_context(tc.tile_pool(name="rel", bufs=3))
                es_pool = att_ctx.enter_context(tc.tile_pool(name="es", bufs=4))
                bc_pool = att_ctx.enter_context(tc.tile_pool(name="bc", bufs=3))
                sm_pool = att_ctx.enter_context(tc.tile_pool(name="sm", bufs=2))

                for h in range(H):
                    oa0 = ps_b.tile([128, 512], F32, tag="psb", name=f"oa0_{h}")
                    oa1 = ps_b.tile([128, 512], F32, tag="psb", name=f"oa1_{h}")
                    for mc in range(NCH):
                        st0 = ps_a.tile([128, 512], F32, tag="psa", name="st0")
                        st1 = ps_a.tile([128, 512], F32, tag="psa", name="st1")
                        kt_sl = kth[:, h, mc * 128:(mc + 1) * 128]
                        rel_t = rel_pool.tile([128, N], F16, tag="rel")
                        nc.gpsimd.dma_start(rel_t[:], rel[h, mc * 128:(mc + 1) * 128, :])
                        nc.tensor.matmul(st0[:], kt_sl, qth[:, h, 0:512], start=True, stop=False)
                        nc.tensor.matmul(st0[:], id_sb[:], rel_t[:, 0:512], start=False, stop=True)
                        nc.tensor.matmul(st1[:], kt_sl, qth[:, h, 512:1024], start=True, stop=False)
                        nc.tensor.matmul(st1[:], id_sb[:], rel_t[:, 512:1024], start=False, stop=True)
                        es = es_pool.tile([128, N], F32R, tag="es")
                        nc.scalar.activation(es[:, 0:512], st0[:], EXP)
                        nc.scalar.activation(es[:, 512:1024], st1[:], EXP)
                        va = vaug[:, mc, h, :]
                        nc.tensor.matmul(oa0[:HD + 1, :], va, es[:, 0:512],
                                         start=(mc == 0), stop=(mc == NCH - 1))
                        nc.tensor.matmul(oa1[:HD + 1, :], va, es[:, 512:1024],
                                         start=(mc == 0), stop=(mc == NCH - 1))
                    # normalize: row HD of oa* holds the softmax denominators per n
                    sums = sm_pool.tile([128, N], F32, tag="sm")
                    nc.scalar.copy(sums[HD:HD + 1, 0:512], oa0[HD:HD + 1, :])
                    nc.scalar.copy(sums[HD:HD + 1, 512:1024], oa1[HD:HD + 1, :])
                    # partition shift 96 -> 0 (DMA), then broadcast on GpSimd
                    nc.sync.dma_start(sums[0:1, :], sums[HD:HD + 1, :])
                    bcb = bc_pool.tile([HD, N], F32, tag="bc")
                    nc.gpsimd.partition_broadcast(bcb[:], sums[0:1, :], channels=HD)
                    # 1/x via DVE Newton-Raphson custom op (~2 ULP)
                    scr = bc_pool.tile([HD, N], F32, tag="bc")
                    rcp = bc_pool.tile([HD, N], F32, tag="bc")
                    nc.vector.reciprocal_approx_accurate(rcp[:, 0:512], bcb[:, 0:512], scr[:, 0:512])
                    nc.vector.reciprocal_approx_accurate(rcp[:, 512:1024], bcb[:, 512:1024], scr[:, 512:1024])
                    nc.vector.tensor_tensor(at_hm[:, h, 0:512], oa0[:HD, :], rcp[:, 0:512], MUL)
                    nc.vector.tensor_tensor(at_hm[:, h, 512:1024], oa1[:HD, :], rcp[:, 512:1024], MUL)

            # ---------------- output projection ----------------
            with ExitStack() as proj_ctx:
                wp_pool = proj_ctx.enter_context(tc.tile_pool(name="wpp", bufs=2))
                ob_pool = proj_ctx.enter_context(tc.tile_pool(name="ob", bufs=3))
                for c0, cw in ((0, 512), (512, 256)):
                    po = [
                        (ps_a if j < 4 else ps_b).tile(
                            [128, 512], F32, tag="psa" if j < 4 else "psb",
                            name=f"po{c0}_{j}")
                        for j in range(NCH)
                    ]
                    for h in range(H):
                        wp_t = wp_pool.tile([HD, 512], F32R, tag="wpp")
                        nc.sync.dma_start(wp_t[:, :cw], wp[:, h, c0:c0 + cw])
                        for j in range(NCH):
                            nc.tensor.matmul(
                                po[j][:, :cw],
                                at_hm[:, h, j * 128:(j + 1) * 128],
                                wp_t[:, :cw],
                                start=(h == 0),
                                stop=False,
                            )
                    for j in range(NCH):
                        # + bias via K=1 ones matmul (broadcast bp over partitions)
                        nc.tensor.matmul(po[j][:, :cw], ones[0:1, :], bp_sb[0:1, c0:c0 + cw],
                                         start=False, stop=True)
                        ot = ob_pool.tile([128, 512], F32, tag="ob")
                        nc.vector.tensor_copy(ot[:, :cw], po[j][:, :cw])
                        nc.scalar.dma_start(out[j * 128:(j + 1) * 128, c0:c0 + cw], ot[:, :cw])

    nc.compile()
    _CACHE["nc"] = nc
    return nc


def make_in_maps(x, y, relative_pos, Wq, Wk, Wv, Wp, bp):
    x = np.asarray(x, dtype=np.float32)
    y = np.asarray(y, dtype=np.float32)
    relative_pos = np.asarray(relative_pos, dtype=np.float32)
    Wq = np.asarray(Wq, dtype=np.float32)
    Wk = np.asarray(Wk, dtype=np.float32)
    Wv = np.asarray(Wv, dtype=np.float32)
    Wp = np.asarray(Wp, dtype=np.float32)
    bp = np.asarray(bp, dtype=np.float32)

    wqT = np.ascontiguousarray(Wq.T)
    wkT = np.ascontiguousarray(Wk.T)
    wvT = np.ascontiguousarray(Wv.T)
    # Wp.T is [c'=h*HD+d, c]; head-major: [d, h, c]
    wp_hm = np.ascontiguousarray(Wp.T.reshape(H, HD, C).transpose(1, 0, 2))
    relT = np.ascontiguousarray(relative_pos.transpose(0, 2, 1)).astype(np.float16)
    bp2 = np.ascontiguousarray(bp.reshape(1, C))

    in_maps = []
    for b in range(B):
        in_maps.append({
            "xT": np.ascontiguousarray(x[b].T),
            "yT": np.ascontiguousarray(y[b].T),
            "wq": wqT, "wk": wkT, "wv": wvT, "wp": wp_hm, "bp": bp2,
            "rel": relT,
            "onesr": np.ones((1, 128), dtype=np.float32),
            "ident": np.eye(128, dtype=np.float16),
            "onesv": np.ones((128, NCH * H), dtype=np.float32),
        })
    return in_maps


def kernel(x, y, relative_pos, H=None, W=None, Wq=None, Wk=None, Wv=None, Wp=None, bp=None,
           **extra):
    nc = build_bass()
    in_maps = make_in_maps(x, y, relative_pos, Wq, Wk, Wv, Wp, bp)
    res = run_bass_kernel_spmd(nc, in_maps, list(range(B)))
    return np.stack([res.results[b]["out"] for b in range(B)], axis=0)


# revision 11
# speedup vs baseline: 1.3901x; 1.1035x over previous
"""Cross-attention Trainium2 kernel (8 NeuronCores, batch-parallel).

Reference computation (per batch element b):
    q = x @ Wq.T ; k = y @ Wk.T ; v = y @ Wv.T          (heads = 8, head_dim = 96)
    S = q k^T * scale + relative_pos                     ([h, n, m])
    out = softmax(S, -1) @ v ; out = out @ Wp.T + bp

Strategy:
  - one batch element per NeuronCore (B == 8 == n_cores), no collectives
  - host-side (free) pre-transposes: xT/yT [C, N]; WqT/WkT/WvT [C, C];
    Wp head-major [HD, H, C]; rel transposed [H, m, n] in fp16
  - on-device: Q.T/K.T head-major [HD, H, N]; V with appended ones column;
    scores computed transposed (S.T[m, n]) so the attention*V matmul needs no
    transposes; the ones column makes the softmax denominator fall out of the
    same matmul (row 96 of out_aug); softmax skips max-subtraction (|S| < ~7,
    exp is safe in fp32)
  - all matmuls in float32r (full PE rate, ~FP22 mantissa)
"""

import numpy as np
from contextlib import ExitStack

import concourse.bass as bass
import concourse.mybir as mybir
import concourse.tile as tile
from concourse import bacc
from concourse.bass_utils import run_bass_kernel_spmd

B, N, C = 8, 1024, 768
H, HD = 8, 96
KCH = C // 128     # 6 contraction chunks
NCH = N // 128     # 8 sequence chunks
SCALE = HD ** -0.5
F32 = mybir.dt.float32
F16 = mybir.dt.float16
F32R = mybir.dt.float32r
ADD = mybir.AluOpType.add
MUL = mybir.AluOpType.mult
EXP = mybir.ActivationFunctionType.Exp
LN = mybir.ActivationFunctionType.Ln

_CACHE = {}


def build_bass():
    if "nc" in _CACHE:
        return _CACHE["nc"]
    nc = bacc.Bacc("TRN2", target_bir_lowering=False, debug=False, num_devices=B)

    xT = nc.dram_tensor("xT", [C, N], F32R, kind="ExternalInput").ap()
    yT = nc.dram_tensor("yT", [C, N], F32R, kind="ExternalInput").ap()
    wq = nc.dram_tensor("wq", [C, C], F32R, kind="ExternalInput").ap()
    wk = nc.dram_tensor("wk", [C, C], F32R, kind="ExternalInput").ap()
    wv = nc.dram_tensor("wv", [C, C], F32R, kind="ExternalInput").ap()
    wp = nc.dram_tensor("wp", [HD, H, C], F32R, kind="ExternalInput").ap()
    bp = nc.dram_tensor("bp", [1, C], F32R, kind="ExternalInput").ap()
    rel = nc.dram_tensor("rel", [H, N, N], F16, kind="ExternalInput").ap()
    onesr = nc.dram_tensor("onesr", [1, 128], F32R, kind="ExternalInput").ap()
    ident = nc.dram_tensor("ident", [128, 128], F16, kind="ExternalInput").ap()
    onesv = nc.dram_tensor("onesv", [128, NCH * H], F32R, kind="ExternalInput").ap()
    out = nc.dram_tensor("out", [N, C], F32, kind="ExternalOutput").ap()

    with tile.TileContext(nc) as tc:
        with ExitStack() as ctx:
            ps_a = ctx.enter_context(tc.tile_pool(name="ps_a", bufs=4, space="PSUM"))
            ps_b = ctx.enter_context(tc.tile_pool(name="ps_b", bufs=4, space="PSUM"))
            qk_pool = ctx.enter_context(tc.tile_pool(name="qk", bufs=2))
            vaug_pool = ctx.enter_context(tc.tile_pool(name="vaug", bufs=1))
            at_pool = ctx.enter_context(tc.tile_pool(name="at", bufs=1))
            const_pool = ctx.enter_context(tc.tile_pool(name="const", bufs=1))

            ones = const_pool.tile([1, 128], F32R)
            nc.sync.dma_start(ones[:], onesr[:])
            bp_sb = const_pool.tile([1, C], F32R)
            nc.sync.dma_start(bp_sb[:], bp[:])
            id_sb = const_pool.tile([128, 128], F16)
            nc.sync.dma_start(id_sb[:], ident[:])

            qth = qk_pool.tile([HD, H, N], F32R, tag="qk", name="qth")
            kth = qk_pool.tile([HD, H, N], F32R, tag="qk", name="kth")
            # V, head-padded, with a ones column at index HD (DMA'd from host;
            # walrus rejects Memset on fp32r)
            vaug = vaug_pool.tile([128, NCH, H, HD + 1], F32R)
            nc.gpsimd.dma_start(vaug[:, :, :, HD], onesv.rearrange("p (a b) -> p a b", a=NCH))
            at_hm = at_pool.tile([HD, H, N], F32R)  # normalized attn-out, head-major

            # ---------------- Q.T / K.T / V projections ----------------
            with ExitStack() as qkv_ctx:
                w_pool = qkv_ctx.enter_context(tc.tile_pool(name="w", bufs=3))
                x_pool = qkv_ctx.enter_context(tc.tile_pool(name="x", bufs=4))
                y_pool = qkv_ctx.enter_context(tc.tile_pool(name="y", bufs=1))

                yt_sb = y_pool.tile([128, KCH, N], F32R)
                nc.gpsimd.dma_start(yt_sb[:], yT.rearrange("(ko ki) n -> ki ko n", ki=128))

                # Q.T and K.T, head-major [HD, H, N]
                for which, w_dram, dst, scale in ((0, wq, qth, SCALE), (1, wk, kth, 1.0)):
                    for nb in range(2):
                        pst = [
                            (ps_a if h < 4 else ps_b).tile(
                                [128, 512], F32, tag="psa" if h < 4 else "psb",
                                name=f"qk{which}_{nb}_{h}")
                            for h in range(H)
                        ]
                        for k in range(KCH):
                            w_t = w_pool.tile([128, C], F32R, tag="w")
                            nc.sync.dma_start(w_t[:], w_dram[k * 128:(k + 1) * 128, :])
                            if which == 0:
                                rhs_t = x_pool.tile([128, 512], F32R, tag="x")
                                nc.sync.dma_start(
                                    rhs_t[:], xT[k * 128:(k + 1) * 128, nb * 512:(nb + 1) * 512]
                                )
                                rhs = rhs_t[:]
                            else:
                                rhs = yt_sb[:, k, nb * 512:(nb + 1) * 512]
                            for h in range(H):
                                nc.tensor.matmul(
                                    pst[h][:HD, :],
                                    w_t[:, h * HD:(h + 1) * HD],
                                    rhs,
                                    start=(k == 0),
                                    stop=(k == KCH - 1),
                                )
                        for h in range(H):
                            if which == 0:
                                nc.scalar.mul(dst[:, h, nb * 512:(nb + 1) * 512], pst[h][:HD, :], scale)
                            else:
                                nc.vector.tensor_copy(dst[:, h, nb * 512:(nb + 1) * 512], pst[h][:HD, :])

                # V (natural [m, c] layout scattered into vaug head slots)
                for c0, cw in ((0, 512), (512, 256)):
                    psv = [
                        (ps_a if mc < 4 else ps_b).tile(
                            [128, 512], F32, tag="psa" if mc < 4 else "psb",
                            name=f"v{c0}_{mc}")
                        for mc in range(NCH)
                    ]
                    for k in range(KCH):
                        w_t = w_pool.tile([128, C], F32R, tag="w")
                        nc.sync.dma_start(w_t[:, :cw], wv[k * 128:(k + 1) * 128, c0:c0 + cw])
                        for mc in range(NCH):
                            nc.tensor.matmul(
                                psv[mc][:, :cw],
                                yt_sb[:, k, mc * 128:(mc + 1) * 128],
                                w_t[:, :cw],
                                start=(k == 0),
                                stop=(k == KCH - 1),
                            )
                    for mc in range(NCH):
                        c = c0
                        i = 0
                        while c < c0 + cw:
                            h = c // HD
                            hi = min((h + 1) * HD, c0 + cw)
                            dst_ap = vaug[:, mc, h, c - h * HD:hi - h * HD]
                            src_ap = psv[mc][:, c - c0:hi - c0]
                            if i % 2 == 0:
                                nc.vector.tensor_copy(dst_ap, src_ap)
                            else:
                                nc.scalar.copy(dst_ap, src_ap)
                            c = hi
                            i += 1

            # ---------------- attention ----------------
            with ExitStack() as att_ctx:
                rel_pool = att_ctx.enter_context(tc.tile_pool(name="rel", bufs=3))
                es_pool = att_ctx.enter_context(tc.tile_pool(name="es", bufs=4))
                bc_pool = att_ctx.enter_context(tc.tile_pool(name="bc", bufs=3))
                sm_pool = att_ctx.enter_context(tc.tile_pool(name="sm", bufs=2))

                for h in range(H):
                    oa0 = ps_b.tile([128, 512], F32, tag="psb", name=f"oa0_{h}")
                    oa1 = ps_b.tile([128, 512], F32, tag="psb", name=f"oa1_{h}")
                    for mc in range(NCH):
                        st0 = ps_a.tile([128, 512], F32, tag="psa", name="st0")
                        st1 = ps_a.tile([128, 512], F32, tag="psa", name="st1")
                        kt_sl = kth[:, h, mc * 128:(mc + 1) * 128]
                        rel_t = rel_pool.tile([128, N], F16, tag="rel")
                        nc.gpsimd.dma_start(rel_t[:], rel[h, mc * 128:(mc + 1) * 128, :])
                        nc.tensor.matmul(st0[:], kt_sl, qth[:, h, 0:512], start=True, stop=False)
                        nc.tensor.matmul(st1[:], kt_sl, qth[:, h, 512:1024], start=True, stop=False)
                        nc.tensor.matmul(st0[:], id_sb[:], rel_t[:, 0:512], start=False, stop=True)
                        nc.tensor.matmul(st1[:], id_sb[:], rel_t[:, 512:1024], start=False, stop=True)
                        es = es_pool.tile([128, N], F32R, tag="es")
                        nc.scalar.activation(es[:, 0:512], st0[:], EXP)
                        nc.scalar.activation(es[:, 512:1024], st1[:], EXP)
                        va = vaug[:, mc, h, :]
                        nc.tensor.matmul(oa0[:HD + 1, :], va, es[:, 0:512],
                                         start=(mc == 0), stop=(mc == NCH - 1))
                        nc.tensor.matmul(oa1[:HD + 1, :], va, es[:, 512:1024],
                                         start=(mc == 0), stop=(mc == NCH - 1))
                    # normalize: row HD of oa* holds the softmax denominators per n
                    sums = sm_pool.tile([128, N], F32, tag="sm")
                    nc.vector.tensor_copy(sums[HD:HD + 1, 0:512], oa0[HD:HD + 1, :])
                    nc.vector.tensor_copy(sums[HD:HD + 1, 512:1024], oa1[HD:HD + 1, :])
                    # partition shift 96 -> 0 via DMA (on HW partition_broadcast
                    # reads physical partition 0 regardless of the AP base)
                    nc.scalar.dma_start(sums[0:1, :], sums[HD:HD + 1, :])
                    bcb = bc_pool.tile([HD, N], F32, tag="bc")
                    nc.gpsimd.partition_broadcast(bcb[:], sums[0:1, :], channels=HD)
                    # 1/x via DVE Newton-Raphson custom op (~2 ULP)
                    scr = bc_pool.tile([HD, N], F32, tag="bc")
                    rcp = bc_pool.tile([HD, N], F32, tag="bc")
                    nc.vector.reciprocal_approx_accurate(rcp[:, 0:512], bcb[:, 0:512], scr[:, 0:512])
                    nc.vector.reciprocal_approx_accurate(rcp[:, 512:1024], bcb[:, 512:1024], scr[:, 512:1024])
                    nc.vector.tensor_tensor(at_hm[:, h, 0:512], oa0[:HD, :], rcp[:, 0:512], MUL)
                    nc.vector.tensor_tensor(at_hm[:, h, 512:1024], oa1[:HD, :], rcp[:, 512:1024], MUL)

            # ---------------- output projection ----------------
            with ExitStack() as proj_ctx:
                wp_pool = proj_ctx.enter_context(tc.tile_pool(name="wpp", bufs=2))
                ob_pool = proj_ctx.enter_context(tc.tile_pool(name="ob", bufs=3))
                for c0, cw in ((0, 512), (512, 256)):
                    po = [
                        (ps_a if j < 4 else ps_b).tile(
                            [128, 512], F32, tag="psa" if j < 4 else "psb",
                            name=f"po{c0}_{j}")
                        for j in range(NCH)
                    ]
                    for h in range(H):
                        wp_t = wp_pool.tile([HD, 512], F32R, tag="wpp")
                        nc.sync.dma_start(wp_t[:, :cw], wp[:, h, c0:c0 + cw])
                        for j in range(NCH):
                            nc.tensor.matmul(
                                po[j][:, :cw],
                                at_hm[:, h, j * 128:(j + 1) * 128],
                                wp_t[:, :cw],
                                start=(h == 0),
                                stop=False,
                            )
                    for j in range(NCH):
                        # + bias via K=1 ones matmul (broadcast bp over partitions)
                        nc.tensor.matmul(po[j][:, :cw], ones[0:1, :], bp_sb[0:1, c0:c0 + cw],
                                         start=False, stop=True)
                        ot = ob_pool.tile([128, 512], F32, tag="ob")
                        nc.vector.tensor_copy(ot[:, :cw], po[j][:, :cw])
                        nc.gpsimd.dma_start(out[j * 128:(j + 1) * 128, c0:c0 + cw], ot[:, :cw])

    nc.compile()
    _CACHE["nc"] = nc
    return nc


def make_in_maps(x, y, relative_pos, Wq, Wk, Wv, Wp, bp):
    x = np.asarray(x, dtype=np.float32)
    y = np.asarray(y, dtype=np.float32)
    relative_pos = np.asarray(relative_pos, dtype=np.float32)
    Wq = np.asarray(Wq, dtype=np.float32)
    Wk = np.asarray(Wk, dtype=np.float32)
    Wv = np.asarray(Wv, dtype=np.float32)
    Wp = np.asarray(Wp, dtype=np.float32)
    bp = np.asarray(bp, dtype=np.float32)

    wqT = np.ascontiguousarray(Wq.T)
    wkT = np.ascontiguousarray(Wk.T)
    wvT = np.ascontiguousarray(Wv.T)
    # Wp.T is [c'=h*HD+d, c]; head-major: [d, h, c]
    wp_hm = np.ascontiguousarray(Wp.T.reshape(H, HD, C).transpose(1, 0, 2))
    relT = np.ascontiguousarray(relative_pos.transpose(0, 2, 1)).astype(np.float16)
    bp2 = np.ascontiguousarray(bp.reshape(1, C))

    in_maps = []
    for b in range(B):
        in_maps.append({
            "xT": np.ascontiguousarray(x[b].T),
            "yT": np.ascontiguousarray(y[b].T),
            "wq": wqT, "wk": wkT, "wv": wvT, "wp": wp_hm, "bp": bp2,
            "rel": relT,
            "onesr": np.ones((1, 128), dtype=np.float32),
            "ident": np.eye(128, dtype=np.float16),
            "onesv": np.ones((128, NCH * H), dtype=np.float32),
        })
    return in_maps


def kernel(x, y, relative_pos, H=None, W=None, Wq=None, Wk=None, Wv=None, Wp=None, bp=None,
           **extra):
    nc = build_bass()
    in_maps = make_in_maps(x, y, relative_pos, Wq, Wk, Wv, Wp, bp)
    res = run_bass_kernel_spmd(nc, in_maps, list(range(B)))
    return np.stack([res.results[b]["out"] for b in range(B)], axis=0)


# revision 12
# speedup vs baseline: 1.4361x; 1.0331x over previous
"""Cross-attention Trainium2 kernel (8 NeuronCores, batch-parallel).

Reference computation (per batch element b):
    q = x @ Wq.T ; k = y @ Wk.T ; v = y @ Wv.T          (heads = 8, head_dim = 96)
    S = q k^T * scale + relative_pos                     ([h, n, m])
    out = softmax(S, -1) @ v ; out = out @ Wp.T + bp

Strategy:
  - one batch element per NeuronCore (B == 8 == n_cores), no collectives
  - host-side (free) pre-transposes: xT/yT [C, N]; WqT/WkT/WvT [C, C];
    Wp head-major [HD, H, C]; rel transposed [H, m, n] in fp16
  - on-device: Q.T/K.T head-major [HD, H, N]; V with appended ones column;
    scores computed transposed (S.T[m, n]) so the attention*V matmul needs no
    transposes; the ones column makes the softmax denominator fall out of the
    same matmul (row 96 of out_aug); softmax skips max-subtraction (|S| < ~7,
    exp is safe in fp32)
  - all matmuls in float32r (full PE rate, ~FP22 mantissa)
"""

import numpy as np
from contextlib import ExitStack

import concourse.bass as bass
import concourse.mybir as mybir
import concourse.tile as tile
from concourse import bacc
from concourse.bass_utils import run_bass_kernel_spmd

B, N, C = 8, 1024, 768
H, HD = 8, 96
KCH = C // 128     # 6 contraction chunks
NCH = N // 128     # 8 sequence chunks
SCALE = HD ** -0.5
F32 = mybir.dt.float32
F16 = mybir.dt.float16
F32R = mybir.dt.float32r
ADD = mybir.AluOpType.add
MUL = mybir.AluOpType.mult
EXP = mybir.ActivationFunctionType.Exp
LN = mybir.ActivationFunctionType.Ln

_CACHE = {}


def build_bass():
    if "nc" in _CACHE:
        return _CACHE["nc"]
    nc = bacc.Bacc("TRN2", target_bir_lowering=False, debug=False, num_devices=B)

    xT = nc.dram_tensor("xT", [C, N], F32R, kind="ExternalInput").ap()
    yT = nc.dram_tensor("yT", [C, N], F32R, kind="ExternalInput").ap()
    wq = nc.dram_tensor("wq", [C, C], F32R, kind="ExternalInput").ap()
    wk = nc.dram_tensor("wk", [C, C], F32R, kind="ExternalInput").ap()
    wv = nc.dram_tensor("wv", [C, C], F32R, kind="ExternalInput").ap()
    wp = nc.dram_tensor("wp", [HD, H, C], F32R, kind="ExternalInput").ap()
    bp = nc.dram_tensor("bp", [1, C], F32R, kind="ExternalInput").ap()
    rel = nc.dram_tensor("rel", [H, N, N], F16, kind="ExternalInput").ap()
    onesr = nc.dram_tensor("onesr", [1, 128], F32R, kind="ExternalInput").ap()
    ident = nc.dram_tensor("ident", [128, 128], F16, kind="ExternalInput").ap()
    onesv = nc.dram_tensor("onesv", [128, NCH * H], F32R, kind="ExternalInput").ap()
    out = nc.dram_tensor("out", [N, C], F32, kind="ExternalOutput").ap()

    with tile.TileContext(nc) as tc:
        with ExitStack() as ctx:
            ps_a = ctx.enter_context(tc.tile_pool(name="ps_a", bufs=4, space="PSUM"))
            ps_b = ctx.enter_context(tc.tile_pool(name="ps_b", bufs=4, space="PSUM"))
            qk_pool = ctx.enter_context(tc.tile_pool(name="qk", bufs=2))
            vaug_pool = ctx.enter_context(tc.tile_pool(name="vaug", bufs=1))
            at_pool = ctx.enter_context(tc.tile_pool(name="at", bufs=1))
            const_pool = ctx.enter_context(tc.tile_pool(name="const", bufs=1))

            ones = const_pool.tile([1, 128], F32R)
            nc.sync.dma_start(ones[:], onesr[:])
            bp_sb = const_pool.tile([1, C], F32R)
            nc.sync.dma_start(bp_sb[:], bp[:])
            id_sb = const_pool.tile([128, 128], F16)
            nc.sync.dma_start(id_sb[:], ident[:])

            qth = qk_pool.tile([HD, H, N], F32R, tag="qk", name="qth")
            kth = qk_pool.tile([HD, H, N], F32R, tag="qk", name="kth")
            # V, head-padded, with a ones column at index HD (DMA'd from host;
            # walrus rejects Memset on fp32r)
            vaug = vaug_pool.tile([128, NCH, H, HD + 1], F32R)
            nc.gpsimd.dma_start(vaug[:, :, :, HD], onesv.rearrange("p (a b) -> p a b", a=NCH))
            at_hm = at_pool.tile([HD, H, N], F32R)  # normalized attn-out, head-major

            # ---------------- Q.T / K.T / V projections ----------------
            with ExitStack() as qkv_ctx:
                w_pool = qkv_ctx.enter_context(tc.tile_pool(name="w", bufs=3))
                x_pool = qkv_ctx.enter_context(tc.tile_pool(name="x", bufs=4))
                y_pool = qkv_ctx.enter_context(tc.tile_pool(name="y", bufs=1))

                yt_sb = y_pool.tile([128, KCH, N], F32R)
                nc.gpsimd.dma_start(yt_sb[:], yT.rearrange("(ko ki) n -> ki ko n", ki=128))

                # Q.T and K.T, head-major [HD, H, N]
                for which, w_dram, dst, scale in ((0, wq, qth, SCALE), (1, wk, kth, 1.0)):
                    for nb in range(2):
                        pst = [
                            (ps_a if h < 4 else ps_b).tile(
                                [128, 512], F32, tag="psa" if h < 4 else "psb",
                                name=f"qk{which}_{nb}_{h}")
                            for h in range(H)
                        ]
                        for k in range(KCH):
                            w_t = w_pool.tile([128, C], F32R, tag="w")
                            nc.sync.dma_start(w_t[:], w_dram[k * 128:(k + 1) * 128, :])
                            if which == 0:
                                rhs_t = x_pool.tile([128, 512], F32R, tag="x")
                                nc.sync.dma_start(
                                    rhs_t[:], xT[k * 128:(k + 1) * 128, nb * 512:(nb + 1) * 512]
                                )
                                rhs = rhs_t[:]
                            else:
                                rhs = yt_sb[:, k, nb * 512:(nb + 1) * 512]
                            for h in range(H):
                                nc.tensor.matmul(
                                    pst[h][:HD, :],
                                    w_t[:, h * HD:(h + 1) * HD],
                                    rhs,
                                    start=(k == 0),
                                    stop=(k == KCH - 1),
                                )
                        for h in range(H):
                            if which == 0:
                                nc.scalar.mul(dst[:, h, nb * 512:(nb + 1) * 512], pst[h][:HD, :], scale)
                            else:
                                nc.vector.tensor_copy(dst[:, h, nb * 512:(nb + 1) * 512], pst[h][:HD, :])

                # V (natural [m, c] layout scattered into vaug head slots)
                for c0, cw in ((0, 512), (512, 256)):
                    psv = [
                        (ps_a if mc < 4 else ps_b).tile(
                            [128, 512], F32, tag="psa" if mc < 4 else "psb",
                            name=f"v{c0}_{mc}")
                        for mc in range(NCH)
                    ]
                    for k in range(KCH):
                        w_t = w_pool.tile([128, C], F32R, tag="w")
                        nc.sync.dma_start(w_t[:, :cw], wv[k * 128:(k + 1) * 128, c0:c0 + cw])
                        for mc in range(NCH):
                            nc.tensor.matmul(
                                psv[mc][:, :cw],
                                yt_sb[:, k, mc * 128:(mc + 1) * 128],
                                w_t[:, :cw],
                                start=(k == 0),
                                stop=(k == KCH - 1),
                            )
                    for mc in range(NCH):
                        c = c0
                        i = 0
                        while c < c0 + cw:
                            h = c // HD
                            hi = min((h + 1) * HD, c0 + cw)
                            dst_ap = vaug[:, mc, h, c - h * HD:hi - h * HD]
                            src_ap = psv[mc][:, c - c0:hi - c0]
                            if i % 2 == 0:
                                nc.vector.tensor_copy(dst_ap, src_ap)
                            else:
                                nc.scalar.copy(dst_ap, src_ap)
                            c = hi
                            i += 1

            # ---------------- attention ----------------
            with ExitStack() as att_ctx:
                rel_pool = att_ctx.enter_context(tc.tile_pool(name="rel", bufs=4))
                es_pool = att_ctx.enter_context(tc.tile_pool(name="es", bufs=4))
                bc_pool = att_ctx.enter_context(tc.tile_pool(name="bc", bufs=3))
                sm_pool = att_ctx.enter_context(tc.tile_pool(name="sm", bufs=2))

                for h in range(H):
                    oa0 = ps_b.tile([128, 512], F32, tag="psb", name=f"oa0_{h}")
                    oa1 = ps_b.tile([128, 512], F32, tag="psb", name=f"oa1_{h}")
                    for mc in range(NCH):
                        st0 = ps_a.tile([128, 512], F32, tag="psa", name="st0")
                        st1 = ps_a.tile([128, 512], F32, tag="psa", name="st1")
                        kt_sl = kth[:, h, mc * 128:(mc + 1) * 128]
                        rel_t = rel_pool.tile([128, N], F16, tag="rel")
                        nc.sync.dma_start(rel_t[:], rel[h, mc * 128:(mc + 1) * 128, :])
                        nc.tensor.matmul(st0[:], kt_sl, qth[:, h, 0:512], start=True, stop=False)
                        nc.tensor.matmul(st1[:], kt_sl, qth[:, h, 512:1024], start=True, stop=False)
                        nc.tensor.matmul(st0[:], id_sb[:], rel_t[:, 0:512], start=False, stop=True)
                        nc.tensor.matmul(st1[:], id_sb[:], rel_t[:, 512:1024], start=False, stop=True)
                        es = es_pool.tile([128, N], F32R, tag="es")
                        nc.scalar.activation(es[:, 0:512], st0[:], EXP)
                        nc.scalar.activation(es[:, 512:1024], st1[:], EXP)
                        va = vaug[:, mc, h, :]
                        nc.tensor.matmul(oa0[:HD + 1, :], va, es[:, 0:512],
                                         start=(mc == 0), stop=(mc == NCH - 1))
                        nc.tensor.matmul(oa1[:HD + 1, :], va, es[:, 512:1024],
                                         start=(mc == 0), stop=(mc == NCH - 1))
                    # normalize: row HD of oa* holds the softmax denominators per n
                    sums = sm_pool.tile([128, N], F32, tag="sm")
                    nc.vector.tensor_copy(sums[HD:HD + 1, 0:512], oa0[HD:HD + 1, :])
                    nc.vector.tensor_copy(sums[HD:HD + 1, 512:1024], oa1[HD:HD + 1, :])
                    # partition shift 96 -> 0 via DMA (on HW partition_broadcast
                    # reads physical partition 0 regardless of the AP base)
                    nc.scalar.dma_start(sums[0:1, :], sums[HD:HD + 1, :])
                    bcb = bc_pool.tile([HD, N], F32, tag="bc")
                    nc.gpsimd.partition_broadcast(bcb[:], sums[0:1, :], channels=HD)
                    # 1/x via DVE Newton-Raphson custom op (~2 ULP)
                    scr = bc_pool.tile([HD, N], F32, tag="bc")
                    rcp = bc_pool.tile([HD, N], F32, tag="bc")
                    nc.vector.reciprocal_approx_accurate(rcp[:, 0:512], bcb[:, 0:512], scr[:, 0:512])
                    nc.vector.reciprocal_approx_accurate(rcp[:, 512:1024], bcb[:, 512:1024], scr[:, 512:1024])
                    nc.vector.tensor_tensor(at_hm[:, h, 0:512], oa0[:HD, :], rcp[:, 0:512], MUL)
                    nc.vector.tensor_tensor(at_hm[:, h, 512:1024], oa1[:HD, :], rcp[:, 512:1024], MUL)

            # ---------------- output projection ----------------
            with ExitStack() as proj_ctx:
                wp_pool = proj_ctx.enter_context(tc.tile_pool(name="wpp", bufs=2))
                ob_pool = proj_ctx.enter_context(tc.tile_pool(name="ob", bufs=3))
                for c0, cw in ((0, 512), (512, 256)):
                    po = [
                        (ps_a if j < 4 else ps_b).tile(
                            [128, 512], F32, tag="psa" if j < 4 else "psb",
                            name=f"po{c0}_{j}")
                        for j in range(NCH)
                    ]
                    for h in range(H):
                        wp_t = wp_pool.tile([HD, 512], F32R, tag="wpp")
                        nc.sync.dma_start(wp_t[:, :cw], wp[:, h, c0:c0 + cw])
                        for j in range(NCH):
                            nc.tensor.matmul(
                                po[j][:, :cw],
                                at_hm[:, h, j * 128:(j + 1) * 128],
                                wp_t[:, :cw],
                                start=(h == 0),
                                stop=False,
                            )
                    for j in range(NCH):
                        # + bias via K=1 ones matmul (broadcast bp over partitions)
                        nc.tensor.matmul(po[j][:, :cw], ones[0:1, :], bp_sb[0:1, c0:c0 + cw],
                                         start=False, stop=True)
                        ot = ob_pool.tile([128, 512], F32, tag="ob")
                        nc.vector.tensor_copy(ot[:, :cw], po[j][:, :cw])
                        nc.gpsimd.dma_start(out[j * 128:(j + 1) * 128, c0:c0 + cw], ot[:, :cw])

    nc.compile()
    _CACHE["nc"] = nc
    return nc


def make_in_maps(x, y, relative_pos, Wq, Wk, Wv, Wp, bp):
    x = np.asarray(x, dtype=np.float32)
    y = np.asarray(y, dtype=np.float32)
    relative_pos = np.asarray(relative_pos, dtype=np.float32)
    Wq = np.asarray(Wq, dtype=np.float32)
    Wk = np.asarray(Wk, dtype=np.float32)
    Wv = np.asarray(Wv, dtype=np.float32)
    Wp = np.asarray(Wp, dtype=np.float32)
    bp = np.asarray(bp, dtype=np.float32)

    wqT = np.ascontiguousarray(Wq.T)
    wkT = np.ascontiguousarray(Wk.T)
    wvT = np.ascontiguousarray(Wv.T)
    # Wp.T is [c'=h*HD+d, c]; head-major: [d, h, c]
    wp_hm = np.ascontiguousarray(Wp.T.reshape(H, HD, C).transpose(1, 0, 2))
    relT = np.ascontiguousarray(relative_pos.transpose(0, 2, 1)).astype(np.float16)
    bp2 = np.ascontiguousarray(bp.reshape(1, C))

    in_maps = []
    for b in range(B):
        in_maps.append({
            "xT": np.ascontiguousarray(x[b].T),
            "yT": np.ascontiguousarray(y[b].T),
            "wq": wqT, "wk": wkT, "wv": wvT, "wp": wp_hm, "bp": bp2,
            "rel": relT,
            "onesr": np.ones((1, 128), dtype=np.float32),
            "ident": np.eye(128, dtype=np.float16),
            "onesv": np.ones((128, NCH * H), dtype=np.float32),
        })
    return in_maps


def kernel(x, y, relative_pos, H=None, W=None, Wq=None, Wk=None, Wv=None, Wp=None, bp=None,
           **extra):
    nc = build_bass()
    in_maps = make_in_maps(x, y, relative_pos, Wq, Wk, Wv, Wp, bp)
    res = run_bass_kernel_spmd(nc, in_maps, list(range(B)))
    return np.stack([res.results[b]["out"] for b in range(B)], axis=0)


# revision 13
# speedup vs baseline: 1.4715x; 1.0246x over previous
"""Cross-attention Trainium2 kernel (8 NeuronCores, batch-parallel).

Reference computation (per batch element b):
    q = x @ Wq.T ; k = y @ Wk.T ; v = y @ Wv.T          (heads = 8, head_dim = 96)
    S = q k^T * scale + relative_pos                     ([h, n, m])
    out = softmax(S, -1) @ v ; out = out @ Wp.T + bp

Strategy:
  - one batch element per NeuronCore (B == 8 == n_cores), no collectives
  - host-side (free) pre-transposes: xT/yT [C, N]; WqT/WkT/WvT [C, C];
    Wp head-major [HD, H, C]; rel transposed [H, m, n] in fp16
  - on-device: Q.T/K.T head-major [HD, H, N]; V with appended ones column;
    scores computed transposed (S.T[m, n]) so the attention*V matmul needs no
    transposes; the ones column makes the softmax denominator fall out of the
    same matmul (row 96 of out_aug); softmax skips max-subtraction (|S| < ~7,
    exp is safe in fp32)
  - all matmuls in float32r (full PE rate, ~FP22 mantissa)
"""

import numpy as np
from contextlib import ExitStack

import concourse.bass as bass
import concourse.mybir as mybir
import concourse.tile as tile
from concourse import bacc
from concourse.bass_utils import run_bass_kernel_spmd

B, N, C = 8, 1024, 768
H, HD = 8, 96
KCH = C // 128     # 6 contraction chunks
NCH = N // 128     # 8 sequence chunks
SCALE = HD ** -0.5
F32 = mybir.dt.float32
F16 = mybir.dt.float16
F32R = mybir.dt.float32r
ADD = mybir.AluOpType.add
MUL = mybir.AluOpType.mult
EXP = mybir.ActivationFunctionType.Exp
LN = mybir.ActivationFunctionType.Ln

_CACHE = {}


def build_bass():
    if "nc" in _CACHE:
        return _CACHE["nc"]
    nc = bacc.Bacc("TRN2", target_bir_lowering=False, debug=False, num_devices=B)

    xT = nc.dram_tensor("xT", [C, N], F32R, kind="ExternalInput").ap()
    yT = nc.dram_tensor("yT", [C, N], F32R, kind="ExternalInput").ap()
    wq = nc.dram_tensor("wq", [C, C], F32R, kind="ExternalInput").ap()
    wk = nc.dram_tensor("wk", [C, C], F32R, kind="ExternalInput").ap()
    wv = nc.dram_tensor("wv", [C, C], F32R, kind="ExternalInput").ap()
    wp = nc.dram_tensor("wp", [HD, H, C], F32R, kind="ExternalInput").ap()
    bp = nc.dram_tensor("bp", [1, C], F32R, kind="ExternalInput").ap()
    rel = nc.dram_tensor("rel", [H, N, N], F16, kind="ExternalInput").ap()
    onesr = nc.dram_tensor("onesr", [1, 128], F32R, kind="ExternalInput").ap()
    ident = nc.dram_tensor("ident", [128, 128], F16, kind="ExternalInput").ap()
    onesv = nc.dram_tensor("onesv", [128, NCH * H], F32R, kind="ExternalInput").ap()
    out = nc.dram_tensor("out", [N, C], F32, kind="ExternalOutput").ap()

    with tile.TileContext(nc) as tc:
        with ExitStack() as ctx:
            ps_a = ctx.enter_context(tc.tile_pool(name="ps_a", bufs=4, space="PSUM"))
            ps_b = ctx.enter_context(tc.tile_pool(name="ps_b", bufs=4, space="PSUM"))
            qk_pool = ctx.enter_context(tc.tile_pool(name="qk", bufs=2))
            vaug_pool = ctx.enter_context(tc.tile_pool(name="vaug", bufs=1))
            at_pool = ctx.enter_context(tc.tile_pool(name="at", bufs=1))
            const_pool = ctx.enter_context(tc.tile_pool(name="const", bufs=1))

            ones = const_pool.tile([1, 128], F32R)
            nc.gpsimd.dma_start(ones[:], onesr[:])
            bp_sb = const_pool.tile([1, C], F32R)
            nc.gpsimd.dma_start(bp_sb[:], bp[:])
            id_sb = const_pool.tile([128, 128], F16)
            nc.gpsimd.dma_start(id_sb[:], ident[:])

            qth = qk_pool.tile([HD, H, N], F32R, tag="qk", name="qth")
            kth = qk_pool.tile([HD, H, N], F32R, tag="qk", name="kth")
            # V, head-padded, with a ones column at index HD (DMA'd from host;
            # walrus rejects Memset on fp32r)
            vaug = vaug_pool.tile([128, NCH, H, HD + 1], F32R)
            nc.gpsimd.dma_start(vaug[:, :, :, HD], onesv.rearrange("p (a b) -> p a b", a=NCH))
            at_hm = at_pool.tile([HD, H, N], F32R)  # normalized attn-out, head-major

            # ---------------- Q.T / K.T / V projections ----------------
            with ExitStack() as qkv_ctx:
                w_pool = qkv_ctx.enter_context(tc.tile_pool(name="w", bufs=3))
                x_pool = qkv_ctx.enter_context(tc.tile_pool(name="x", bufs=4))
                y_pool = qkv_ctx.enter_context(tc.tile_pool(name="y", bufs=1))

                yt_sb = y_pool.tile([128, KCH, N], F32R)
                nc.gpsimd.dma_start(yt_sb[:], yT.rearrange("(ko ki) n -> ki ko n", ki=128))

                # Q.T and K.T, head-major [HD, H, N]
                for which, w_dram, dst, scale in ((0, wq, qth, SCALE), (1, wk, kth, 1.0)):
                    for nb in range(2):
                        pst = [
                            (ps_a if h < 4 else ps_b).tile(
                                [128, 512], F32, tag="psa" if h < 4 else "psb",
                                name=f"qk{which}_{nb}_{h}")
                            for h in range(H)
                        ]
                        for k in range(KCH):
                            w_t = w_pool.tile([128, C], F32R, tag="w")
                            weng = nc.sync if k % 2 == 0 else nc.scalar
                            weng.dma_start(w_t[:], w_dram[k * 128:(k + 1) * 128, :])
                            if which == 0:
                                rhs_t = x_pool.tile([128, 512], F32R, tag="x")
                                xeng = nc.scalar if k % 2 == 0 else nc.sync
                                xeng.dma_start(
                                    rhs_t[:], xT[k * 128:(k + 1) * 128, nb * 512:(nb + 1) * 512]
                                )
                                rhs = rhs_t[:]
                            else:
                                rhs = yt_sb[:, k, nb * 512:(nb + 1) * 512]
                            for h in range(H):
                                nc.tensor.matmul(
                                    pst[h][:HD, :],
                                    w_t[:, h * HD:(h + 1) * HD],
                                    rhs,
                                    start=(k == 0),
                                    stop=(k == KCH - 1),
                                )
                        for h in range(H):
                            if which == 0:
                                nc.scalar.mul(dst[:, h, nb * 512:(nb + 1) * 512], pst[h][:HD, :], scale)
                            else:
                                nc.vector.tensor_copy(dst[:, h, nb * 512:(nb + 1) * 512], pst[h][:HD, :])

                # V (natural [m, c] layout scattered into vaug head slots)
                for c0, cw in ((0, 512), (512, 256)):
                    psv = [
                        (ps_a if mc < 4 else ps_b).tile(
                            [128, 512], F32, tag="psa" if mc < 4 else "psb",
                            name=f"v{c0}_{mc}")
                        for mc in range(NCH)
                    ]
                    for k in range(KCH):
                        w_t = w_pool.tile([128, C], F32R, tag="w")
                        weng = nc.sync if k % 2 == 0 else nc.scalar
                        weng.dma_start(w_t[:, :cw], wv[k * 128:(k + 1) * 128, c0:c0 + cw])
                        for mc in range(NCH):
                            nc.tensor.matmul(
                                psv[mc][:, :cw],
                                yt_sb[:, k, mc * 128:(mc + 1) * 128],
                                w_t[:, :cw],
                                start=(k == 0),
                                stop=(k == KCH - 1),
                            )
                    for mc in range(NCH):
                        c = c0
                        i = 0
                        while c < c0 + cw:
                            h = c // HD
                            hi = min((h + 1) * HD, c0 + cw)
                            dst_ap = vaug[:, mc, h, c - h * HD:hi - h * HD]
                            src_ap = psv[mc][:, c - c0:hi - c0]
                            if i % 2 == 0:
                                nc.vector.tensor_copy(dst_ap, src_ap)
                            else:
                                nc.scalar.copy(dst_ap, src_ap)
                            c = hi
                            i += 1

            # ---------------- attention ----------------
            with ExitStack() as att_ctx:
                rel_pool = att_ctx.enter_context(tc.tile_pool(name="rel", bufs=4))
                es_pool = att_ctx.enter_context(tc.tile_pool(name="es", bufs=4))
                bc_pool = att_ctx.enter_context(tc.tile_pool(name="bc", bufs=3))
                sm_pool = att_ctx.enter_context(tc.tile_pool(name="sm", bufs=2))

                for h in range(H):
                    oa0 = ps_b.tile([128, 512], F32, tag="psb", name=f"oa0_{h}")
                    oa1 = ps_b.tile([128, 512], F32, tag="psb", name=f"oa1_{h}")
                    for mc in range(NCH):
                        st0 = ps_a.tile([128, 512], F32, tag="psa", name="st0")
                        st1 = ps_a.tile([128, 512], F32, tag="psa", name="st1")
                        kt_sl = kth[:, h, mc * 128:(mc + 1) * 128]
                        rel_t = rel_pool.tile([128, N], F16, tag="rel")
                        (nc.sync if mc % 2 == 0 else nc.gpsimd).dma_start(rel_t[:], rel[h, mc * 128:(mc + 1) * 128, :])
                        nc.tensor.matmul(st0[:], kt_sl, qth[:, h, 0:512], start=True, stop=False)
                        nc.tensor.matmul(st1[:], kt_sl, qth[:, h, 512:1024], start=True, stop=False)
                        nc.tensor.matmul(st0[:], id_sb[:], rel_t[:, 0:512], start=False, stop=True)
                        nc.tensor.matmul(st1[:], id_sb[:], rel_t[:, 512:1024], start=False, stop=True)
                        es = es_pool.tile([128, N], F32R, tag="es")
                        nc.scalar.activation(es[:, 0:512], st0[:], EXP)
                        nc.scalar.activation(es[:, 512:1024], st1[:], EXP)
                        va = vaug[:, mc, h, :]
                        nc.tensor.matmul(oa0[:HD + 1, :], va, es[:, 0:512],
                                         start=(mc == 0), stop=(mc == NCH - 1))
                        nc.tensor.matmul(oa1[:HD + 1, :], va, es[:, 512:1024],
                                         start=(mc == 0), stop=(mc == NCH - 1))
                    # normalize: row HD of oa* holds the softmax denominators per n
                    sums = sm_pool.tile([128, N], F32, tag="sm")
                    nc.vector.tensor_copy(sums[HD:HD + 1, 0:512], oa0[HD:HD + 1, :])
                    nc.vector.tensor_copy(sums[HD:HD + 1, 512:1024], oa1[HD:HD + 1, :])
                    # partition shift 96 -> 0 via DMA (on HW partition_broadcast
                    # reads physical partition 0 regardless of the AP base)
                    nc.scalar.dma_start(sums[0:1, :], sums[HD:HD + 1, :])
                    bcb = bc_pool.tile([HD, N], F32, tag="bc")
                    nc.gpsimd.partition_broadcast(bcb[:], sums[0:1, :], channels=HD)
                    # 1/x via DVE Newton-Raphson custom op (~2 ULP)
                    scr = bc_pool.tile([HD, N], F32, tag="bc")
                    rcp = bc_pool.tile([HD, N], F32, tag="bc")
                    nc.vector.reciprocal_approx_accurate(rcp[:, 0:512], bcb[:, 0:512], scr[:, 0:512])
                    nc.vector.reciprocal_approx_accurate(rcp[:, 512:1024], bcb[:, 512:1024], scr[:, 512:1024])
                    nc.vector.tensor_tensor(at_hm[:, h, 0:512], oa0[:HD, :], rcp[:, 0:512], MUL)
                    nc.vector.tensor_tensor(at_hm[:, h, 512:1024], oa1[:HD, :], rcp[:, 512:1024], MUL)

            # ---------------- output projection ----------------
            with ExitStack() as proj_ctx:
                wp_pool = proj_ctx.enter_context(tc.tile_pool(name="wpp", bufs=2))
                ob_pool = proj_ctx.enter_context(tc.tile_pool(name="ob", bufs=3))
                for c0, cw in ((0, 512), (512, 256)):
                    po = [
                        (ps_a if j < 4 else ps_b).tile(
                            [128, 512], F32, tag="psa" if j < 4 else "psb",
                            name=f"po{c0}_{j}")
                        for j in range(NCH)
                    ]
                    for h in range(H):
                        wp_t = wp_pool.tile([HD, 512], F32R, tag="wpp")
                        (nc.sync if h % 2 == 0 else nc.gpsimd).dma_start(wp_t[:, :cw], wp[:, h, c0:c0 + cw])
                        for j in range(NCH):
                            nc.tensor.matmul(
                                po[j][:, :cw],
                                at_hm[:, h, j * 128:(j + 1) * 128],
                                wp_t[:, :cw],
                                start=(h == 0),
                                stop=False,
                            )
                    for j in range(NCH):
                        # + bias via K=1 ones matmul (broadcast bp over partitions)
                        nc.tensor.matmul(po[j][:, :cw], ones[0:1, :], bp_sb[0:1, c0:c0 + cw],
                                         start=False, stop=True)
                        ot = ob_pool.tile([128, 512], F32, tag="ob")
                        nc.vector.tensor_copy(ot[:, :cw], po[j][:, :cw])
                        nc.scalar.dma_start(out[j * 128:(j + 1) * 128, c0:c0 + cw], ot[:, :cw])

    nc.compile()
    _CACHE["nc"] = nc
    return nc


def make_in_maps(x, y, relative_pos, Wq, Wk, Wv, Wp, bp):
    x = np.asarray(x, dtype=np.float32)
    y = np.asarray(y, dtype=np.float32)
    relative_pos = np.asarray(relative_pos, dtype=np.float32)
    Wq = np.asarray(Wq, dtype=np.float32)
    Wk = np.asarray(Wk, dtype=np.float32)
    Wv = np.asarray(Wv, dtype=np.float32)
    Wp = np.asarray(Wp, dtype=np.float32)
    bp = np.asarray(bp, dtype=np.float32)

    wqT = np.ascontiguousarray(Wq.T)
    wkT = np.ascontiguousarray(Wk.T)
    wvT = np.ascontiguousarray(Wv.T)
    # Wp.T is [c'=h*HD+d, c]; head-major: [d, h, c]
    wp_hm = np.ascontiguousarray(Wp.T.reshape(H, HD, C).transpose(1, 0, 2))
    relT = np.ascontiguousarray(relative_pos.transpose(0, 2, 1)).astype(np.float16)
    bp2 = np.ascontiguousarray(bp.reshape(1, C))

    in_maps = []
    for b in range(B):
        in_maps.append({
            "xT": np.ascontiguousarray(x[b].T),
            "yT": np.ascontiguousarray(y[b].T),
            "wq": wqT, "wk": wkT, "wv": wvT, "wp": wp_hm, "bp": bp2,
            "rel": relT,
            "onesr": np.ones((1, 128), dtype=np.float32),
            "ident": np.eye(128, dtype=np.float16),
            "onesv": np.ones((128, NCH * H), dtype=np.float32),
        })
    return in_maps


def kernel(x, y, relative_pos, H=None, W=None, Wq=None, Wk=None, Wv=None, Wp=None, bp=None,
           **extra):
    nc = build_bass()
    in_maps = make_in_maps(x, y, relative_pos, Wq, Wk, Wv, Wp, bp)
    res = run_bass_kernel_spmd(nc, in_maps, list(range(B)))
    return np.stack([res.results[b]["out"] for b in range(B)], axis=0)


# revision 14
# speedup vs baseline: 1.5433x; 1.0488x over previous
"""Cross-attention Trainium2 kernel (8 NeuronCores, batch-parallel).

Reference computation (per batch element b):
    q = x @ Wq.T ; k = y @ Wk.T ; v = y @ Wv.T          (heads = 8, head_dim = 96)
    S = q k^T * scale + relative_pos                     ([h, n, m])
    out = softmax(S, -1) @ v ; out = out @ Wp.T + bp

Strategy:
  - one batch element per NeuronCore (B == 8 == n_cores), no collectives
  - host-side (free) pre-transposes: xT/yT [C, N]; WqT/WkT/WvT [C, C];
    Wp head-major [HD, H, C]; rel transposed [H, m, n] in fp16
  - on-device: Q.T/K.T head-major [HD, H, N]; V with appended ones column;
    scores computed transposed (S.T[m, n]) so the attention*V matmul needs no
    transposes; the ones column makes the softmax denominator fall out of the
    same matmul (row 96 of out_aug); softmax skips max-subtraction (|S| < ~7,
    exp is safe in fp32)
  - all matmuls in float32r (full PE rate, ~FP22 mantissa)
"""

import numpy as np
from contextlib import ExitStack

import concourse.bass as bass
import concourse.mybir as mybir
import concourse.tile as tile
from concourse import bacc
from concourse.bass_utils import run_bass_kernel_spmd

B, N, C = 8, 1024, 768
H, HD = 8, 96
KCH = C // 128     # 6 contraction chunks
NCH = N // 128     # 8 sequence chunks
SCALE = HD ** -0.5
F32 = mybir.dt.float32
F16 = mybir.dt.float16
F32R = mybir.dt.float32r
ADD = mybir.AluOpType.add
MUL = mybir.AluOpType.mult
EXP = mybir.ActivationFunctionType.Exp
LN = mybir.ActivationFunctionType.Ln

_CACHE = {}


def build_bass():
    if "nc" in _CACHE:
        return _CACHE["nc"]
    nc = bacc.Bacc("TRN2", target_bir_lowering=False, debug=False, num_devices=B)

    xT = nc.dram_tensor("xT", [C, N], F32R, kind="ExternalInput").ap()
    yT = nc.dram_tensor("yT", [C, N], F32R, kind="ExternalInput").ap()
    wq = nc.dram_tensor("wq", [C, C], F32R, kind="ExternalInput").ap()
    wk = nc.dram_tensor("wk", [C, C], F32R, kind="ExternalInput").ap()
    wv = nc.dram_tensor("wv", [C, C], F32R, kind="ExternalInput").ap()
    wp = nc.dram_tensor("wp", [HD, H, C], F32R, kind="ExternalInput").ap()
    bp = nc.dram_tensor("bp", [1, C], F32R, kind="ExternalInput").ap()
    rel = nc.dram_tensor("rel", [H, N, N], F16, kind="ExternalInput").ap()
    onesr = nc.dram_tensor("onesr", [1, 128], F32R, kind="ExternalInput").ap()
    ident = nc.dram_tensor("ident", [128, 128], F16, kind="ExternalInput").ap()
    onesv = nc.dram_tensor("onesv", [128, NCH * H], F32R, kind="ExternalInput").ap()
    out = nc.dram_tensor("out", [N, C], F32, kind="ExternalOutput").ap()

    with tile.TileContext(nc) as tc:
        with ExitStack() as ctx:
            ps_a = ctx.enter_context(tc.tile_pool(name="ps_a", bufs=4, space="PSUM"))
            ps_b = ctx.enter_context(tc.tile_pool(name="ps_b", bufs=4, space="PSUM"))
            qk_pool = ctx.enter_context(tc.tile_pool(name="qk", bufs=2))
            vaug_pool = ctx.enter_context(tc.tile_pool(name="vaug", bufs=1))
            at_pool = ctx.enter_context(tc.tile_pool(name="at", bufs=1))
            const_pool = ctx.enter_context(tc.tile_pool(name="const", bufs=1))

            ones = const_pool.tile([1, 128], F32R)
            nc.gpsimd.dma_start(ones[:], onesr[:])
            bp_sb = const_pool.tile([1, C], F32R)
            nc.gpsimd.dma_start(bp_sb[:], bp[:])
            id_sb = const_pool.tile([128, 128], F16)
            nc.gpsimd.dma_start(id_sb[:], ident[:])

            qth = qk_pool.tile([HD, H, N], F32R, tag="qk", name="qth")
            kth = qk_pool.tile([HD, H, N], F32R, tag="qk", name="kth")
            # V, head-padded, with a ones column at index HD (DMA'd from host;
            # walrus rejects Memset on fp32r)
            vaug = vaug_pool.tile([128, NCH, H, HD + 1], F32R)
            nc.gpsimd.dma_start(vaug[:, :, :, HD], onesv.rearrange("p (a b) -> p a b", a=NCH))
            at_hm = at_pool.tile([HD, H, N], F32R)  # normalized attn-out, head-major

            # ---------------- Q.T / K.T / V projections ----------------
            with ExitStack() as qkv_ctx:
                w_pool = qkv_ctx.enter_context(tc.tile_pool(name="w", bufs=6))
                x_pool = qkv_ctx.enter_context(tc.tile_pool(name="x", bufs=8))
                y_pool = qkv_ctx.enter_context(tc.tile_pool(name="y", bufs=1))

                yt_sb = y_pool.tile([128, KCH, N], F32R)
                nc.gpsimd.dma_start(yt_sb[:], yT.rearrange("(ko ki) n -> ki ko n", ki=128))

                # Q.T and K.T, head-major [HD, H, N]
                for which, w_dram, dst, scale in ((0, wq, qth, SCALE), (1, wk, kth, 1.0)):
                    for nb in range(2):
                        pst = [
                            (ps_a if h < 4 else ps_b).tile(
                                [128, 512], F32, tag="psa" if h < 4 else "psb",
                                name=f"qk{which}_{nb}_{h}")
                            for h in range(H)
                        ]
                        for k in range(KCH):
                            w_t = w_pool.tile([128, C], F32R, tag="w")
                            weng = nc.sync if k % 2 == 0 else nc.scalar
                            weng.dma_start(w_t[:], w_dram[k * 128:(k + 1) * 128, :])
                            if which == 0:
                                rhs_t = x_pool.tile([128, 512], F32R, tag="x")
                                xeng = nc.scalar if k % 2 == 0 else nc.sync
                                xeng.dma_start(
                                    rhs_t[:], xT[k * 128:(k + 1) * 128, nb * 512:(nb + 1) * 512]
                                )
                                rhs = rhs_t[:]
                            else:
                                rhs = yt_sb[:, k, nb * 512:(nb + 1) * 512]
                            for h in range(H):
                                nc.tensor.matmul(
                                    pst[h][:HD, :],
                                    w_t[:, h * HD:(h + 1) * HD],
                                    rhs,
                                    start=(k == 0),
                                    stop=(k == KCH - 1),
                                )
                        for h in range(H):
                            if which == 0:
                                nc.scalar.mul(dst[:, h, nb * 512:(nb + 1) * 512], pst[h][:HD, :], scale)
                            else:
                                nc.vector.tensor_copy(dst[:, h, nb * 512:(nb + 1) * 512], pst[h][:HD, :])

                # V (natural [m, c] layout scattered into vaug head slots)
                for c0, cw in ((0, 512), (512, 256)):
                    psv = [
                        (ps_a if mc < 4 else ps_b).tile(
                            [128, 512], F32, tag="psa" if mc < 4 else "psb",
                            name=f"v{c0}_{mc}")
                        for mc in range(NCH)
                    ]
                    for k in range(KCH):
                        w_t = w_pool.tile([128, C], F32R, tag="w")
                        weng = nc.sync if k % 2 == 0 else nc.scalar
                        weng.dma_start(w_t[:, :cw], wv[k * 128:(k + 1) * 128, c0:c0 + cw])
                        for mc in range(NCH):
                            nc.tensor.matmul(
                                psv[mc][:, :cw],
                                yt_sb[:, k, mc * 128:(mc + 1) * 128],
                                w_t[:, :cw],
                                start=(k == 0),
                                stop=(k == KCH - 1),
                            )
                    for mc in range(NCH):
                        c = c0
                        i = 0
                        while c < c0 + cw:
                            h = c // HD
                            hi = min((h + 1) * HD, c0 + cw)
                            dst_ap = vaug[:, mc, h, c - h * HD:hi - h * HD]
                            src_ap = psv[mc][:, c - c0:hi - c0]
                            if i % 2 == 0:
                                nc.vector.tensor_copy(dst_ap, src_ap)
                            else:
                                nc.scalar.copy(dst_ap, src_ap)
                            c = hi
                            i += 1

            # ---------------- attention ----------------
            with ExitStack() as att_ctx:
                rel_pool = att_ctx.enter_context(tc.tile_pool(name="rel", bufs=4))
                es_pool = att_ctx.enter_context(tc.tile_pool(name="es", bufs=4))
                bc_pool = att_ctx.enter_context(tc.tile_pool(name="bc", bufs=3))
                sm_pool = att_ctx.enter_context(tc.tile_pool(name="sm", bufs=2))

                for h in range(H):
                    oa0 = ps_b.tile([128, 512], F32, tag="psb", name=f"oa0_{h}")
                    oa1 = ps_b.tile([128, 512], F32, tag="psb", name=f"oa1_{h}")
                    for mc in range(NCH):
                        st0 = ps_a.tile([128, 512], F32, tag="psa", name="st0")
                        st1 = ps_a.tile([128, 512], F32, tag="psa", name="st1")
                        kt_sl = kth[:, h, mc * 128:(mc + 1) * 128]
                        rel_t = rel_pool.tile([128, N], F16, tag="rel")
                        (nc.sync if mc % 2 == 0 else nc.gpsimd).dma_start(rel_t[:], rel[h, mc * 128:(mc + 1) * 128, :])
                        nc.tensor.matmul(st0[:], kt_sl, qth[:, h, 0:512], start=True, stop=False)
                        nc.tensor.matmul(st1[:], kt_sl, qth[:, h, 512:1024], start=True, stop=False)
                        nc.tensor.matmul(st0[:], id_sb[:], rel_t[:, 0:512], start=False, stop=True)
                        nc.tensor.matmul(st1[:], id_sb[:], rel_t[:, 512:1024], start=False, stop=True)
                        es = es_pool.tile([128, N], F32R, tag="es")
                        nc.scalar.activation(es[:, 0:512], st0[:], EXP)
                        nc.scalar.activation(es[:, 512:1024], st1[:], EXP)
                        va = vaug[:, mc, h, :]
                        nc.tensor.matmul(oa0[:HD + 1, :], va, es[:, 0:512],
                                         start=(mc == 0), stop=(mc == NCH - 1))
                        nc.tensor.matmul(oa1[:HD + 1, :], va, es[:, 512:1024],
                                         start=(mc == 0), stop=(mc == NCH - 1))
                    # normalize: row HD of oa* holds the softmax denominators per n
                    sums = sm_pool.tile([128, N], F32, tag="sm")
                    nc.vector.tensor_copy(sums[HD:HD + 1, 0:512], oa0[HD:HD + 1, :])
                    nc.vector.tensor_copy(sums[HD:HD + 1, 512:1024], oa1[HD:HD + 1, :])
                    # partition shift 96 -> 0 via DMA (on HW partition_broadcast
                    # reads physical partition 0 regardless of the AP base)
                    nc.scalar.dma_start(sums[0:1, :], sums[HD:HD + 1, :])
                    bcb = bc_pool.tile([HD, N], F32, tag="bc")
                    nc.gpsimd.partition_broadcast(bcb[:], sums[0:1, :], channels=HD)
                    # 1/x via DVE Newton-Raphson custom op (~2 ULP)
                    scr = bc_pool.tile([HD, N], F32, tag="bc")
                    rcp = bc_pool.tile([HD, N], F32, tag="bc")
                    nc.vector.reciprocal_approx_accurate(rcp[:, 0:512], bcb[:, 0:512], scr[:, 0:512])
                    nc.vector.reciprocal_approx_accurate(rcp[:, 512:1024], bcb[:, 512:1024], scr[:, 512:1024])
                    nc.vector.tensor_tensor(at_hm[:, h, 0:512], oa0[:HD, :], rcp[:, 0:512], MUL)
                    nc.vector.tensor_tensor(at_hm[:, h, 512:1024], oa1[:HD, :], rcp[:, 512:1024], MUL)

            # ---------------- output projection ----------------
            with ExitStack() as proj_ctx:
                wp_pool = proj_ctx.enter_context(tc.tile_pool(name="wpp", bufs=4))
                ob_pool = proj_ctx.enter_context(tc.tile_pool(name="ob", bufs=3))
                for c0, cw in ((0, 512), (512, 256)):
                    po = [
                        (ps_a if j < 4 else ps_b).tile(
                            [128, 512], F32, tag="psa" if j < 4 else "psb",
                            name=f"po{c0}_{j}")
                        for j in range(NCH)
                    ]
                    for h in range(H):
                        wp_t = wp_pool.tile([HD, 512], F32R, tag="wpp")
                        (nc.sync if h % 2 == 0 else nc.gpsimd).dma_start(wp_t[:, :cw], wp[:, h, c0:c0 + cw])
                        for j in range(NCH):
                            nc.tensor.matmul(
                                po[j][:, :cw],
                                at_hm[:, h, j * 128:(j + 1) * 128],
                                wp_t[:, :cw],
                                start=(h == 0),
                                stop=False,
                            )
                    for j in range(NCH):
                        # + bias via K=1 ones matmul (broadcast bp over partitions)
                        nc.tensor.matmul(po[j][:, :cw], ones[0:1, :], bp_sb[0:1, c0:c0 + cw],
                                         start=False, stop=True)
                        ot = ob_pool.tile([128, 512], F32, tag="ob")
                        nc.vector.tensor_copy(ot[:, :cw], po[j][:, :cw])
                        nc.scalar.dma_start(out[j * 128:(j + 1) * 128, c0:c0 + cw], ot[:, :cw])

    nc.compile()
    _CACHE["nc"] = nc
    return nc


def make_in_maps(x, y, relative_pos, Wq, Wk, Wv, Wp, bp):
    x = np.asarray(x, dtype=np.float32)
    y = np.asarray(y, dtype=np.float32)
    relative_pos = np.asarray(relative_pos, dtype=np.float32)
    Wq = np.asarray(Wq, dtype=np.float32)
    Wk = np.asarray(Wk, dtype=np.float32)
    Wv = np.asarray(Wv, dtype=np.float32)
    Wp = np.asarray(Wp, dtype=np.float32)
    bp = np.asarray(bp, dtype=np.float32)

    wqT = np.ascontiguousarray(Wq.T)
    wkT = np.ascontiguousarray(Wk.T)
    wvT = np.ascontiguousarray(Wv.T)
    # Wp.T is [c'=h*HD+d, c]; head-major: [d, h, c]
    wp_hm = np.ascontiguousarray(Wp.T.reshape(H, HD, C).transpose(1, 0, 2))
    relT = np.ascontiguousarray(relative_pos.transpose(0, 2, 1)).astype(np.float16)
    bp2 = np.ascontiguousarray(bp.reshape(1, C))

    in_maps = []
    for b in range(B):
        in_maps.append({
            "xT": np.ascontiguousarray(x[b].T),
            "yT": np.ascontiguousarray(y[b].T),
            "wq": wqT, "wk": wkT, "wv": wvT, "wp": wp_hm, "bp": bp2,
            "rel": relT,
            "onesr": np.ones((1, 128), dtype=np.float32),
            "ident": np.eye(128, dtype=np.float16),
            "onesv": np.ones((128, NCH * H), dtype=np.float32),
        })
    return in_maps


def kernel(x, y, relative_pos, H=None, W=None, Wq=None, Wk=None, Wv=None, Wp=None, bp=None,
           **extra):
    nc = build_bass()
    in_maps = make_in_maps(x, y, relative_pos, Wq, Wk, Wv, Wp, bp)
    res = run_bass_kernel_spmd(nc, in_maps, list(range(B)))
    return np.stack([res.results[b]["out"] for b in range(B)], axis=0)


# revision 15
# speedup vs baseline: 1.8130x; 1.1747x over previous
"""Cross-attention Trainium2 kernel (8 NeuronCores, batch-parallel).

Reference computation (per batch element b):
    q = x @ Wq.T ; k = y @ Wk.T ; v = y @ Wv.T          (heads = 8, head_dim = 96)
    S = q k^T * scale + relative_pos                     ([h, n, m])
    out = softmax(S, -1) @ v ; out = out @ Wp.T + bp

Strategy:
  - one batch element per NeuronCore (B == 8 == n_cores), no collectives
  - host-side (free) pre-transposes: xT/yT [C, N]; WqT/WkT/WvT [C, C];
    Wp head-major [HD, H, C]; rel transposed [H, m, n] in fp16
  - on-device: Q.T/K.T head-major [HD, H, N]; V with appended ones column;
    scores computed transposed (S.T[m, n]) so the attention*V matmul needs no
    transposes; the ones column makes the softmax denominator fall out of the
    same matmul (row 96 of out_aug); softmax skips max-subtraction (|S| < ~7,
    exp is safe in fp32)
  - all matmuls in float32r (full PE rate, ~FP22 mantissa)
"""

import numpy as np
from contextlib import ExitStack

import concourse.bass as bass
import concourse.mybir as mybir
import concourse.tile as tile
from concourse import bacc
from concourse.bass_utils import run_bass_kernel_spmd

B, N, C = 8, 1024, 768
H, HD = 8, 96
KCH = C // 128     # 6 contraction chunks
NCH = N // 128     # 8 sequence chunks
SCALE = HD ** -0.5
F32 = mybir.dt.float32
F16 = mybir.dt.float16
F32R = mybir.dt.float32r
ADD = mybir.AluOpType.add
MUL = mybir.AluOpType.mult
EXP = mybir.ActivationFunctionType.Exp
LN = mybir.ActivationFunctionType.Ln

_CACHE = {}


def build_bass():
    if "nc" in _CACHE:
        return _CACHE["nc"]
    nc = bacc.Bacc("TRN2", target_bir_lowering=False, debug=False, num_devices=B)

    xT = nc.dram_tensor("xT", [C, N], F32R, kind="ExternalInput").ap()
    yT = nc.dram_tensor("yT", [C, N], F32R, kind="ExternalInput").ap()
    wq = nc.dram_tensor("wq", [C, C], F32R, kind="ExternalInput").ap()
    wk = nc.dram_tensor("wk", [C, C], F32R, kind="ExternalInput").ap()
    wv = nc.dram_tensor("wv", [C, C], F32R, kind="ExternalInput").ap()
    wp = nc.dram_tensor("wp", [HD, H, C], F32R, kind="ExternalInput").ap()
    bp = nc.dram_tensor("bp", [1, C], F32R, kind="ExternalInput").ap()
    rel = nc.dram_tensor("rel", [H, N, N], F16, kind="ExternalInput").ap()
    onesr = nc.dram_tensor("onesr", [1, 128], F32R, kind="ExternalInput").ap()
    ident = nc.dram_tensor("ident", [128, 128], F16, kind="ExternalInput").ap()
    onesv = nc.dram_tensor("onesv", [128, NCH * H], F32R, kind="ExternalInput").ap()
    out = nc.dram_tensor("out", [N, C], F32, kind="ExternalOutput").ap()

    with tile.TileContext(nc) as tc:
        with ExitStack() as ctx:
            ps_a = ctx.enter_context(tc.tile_pool(name="ps_a", bufs=4, space="PSUM"))
            ps_b = ctx.enter_context(tc.tile_pool(name="ps_b", bufs=4, space="PSUM"))
            qk_pool = ctx.enter_context(tc.tile_pool(name="qk", bufs=2))
            vaug_pool = ctx.enter_context(tc.tile_pool(name="vaug", bufs=1))
            at_pool = ctx.enter_context(tc.tile_pool(name="at", bufs=1))
            const_pool = ctx.enter_context(tc.tile_pool(name="const", bufs=1))

            ones = const_pool.tile([1, 128], F32R)
            nc.gpsimd.dma_start(ones[:], onesr[:])
            bp_sb = const_pool.tile([1, C], F32R)
            nc.gpsimd.dma_start(bp_sb[:], bp[:])
            id_sb = const_pool.tile([128, 128], F16)
            nc.gpsimd.dma_start(id_sb[:], ident[:])

            qth = qk_pool.tile([HD, H, N], F32R, tag="qk", name="qth")
            kth = qk_pool.tile([HD, H, N], F32R, tag="qk", name="kth")
            # V, head-padded, with a ones column at index HD (DMA'd from host;
            # walrus rejects Memset on fp32r)
            vaug = vaug_pool.tile([128, NCH, H, HD + 1], F32R)
            nc.gpsimd.dma_start(vaug[:, :, :, HD], onesv.rearrange("p (a b) -> p a b", a=NCH))
            at_hm = at_pool.tile([HD, H, N], F32R)  # normalized attn-out, head-major

            # ---------------- Q.T / K.T / V projections ----------------
            with ExitStack() as qkv_ctx:
                w_pool = qkv_ctx.enter_context(tc.tile_pool(name="w", bufs=2))
                x_pool = qkv_ctx.enter_context(tc.tile_pool(name="x", bufs=8))
                y_pool = qkv_ctx.enter_context(tc.tile_pool(name="y", bufs=1))

                yt_sb = y_pool.tile([128, KCH, N], F32R)
                nc.gpsimd.dma_start(yt_sb[:], yT.rearrange("(ko ki) n -> ki ko n", ki=128))

                # Q.T and K.T, head-major [HD, H, N]
                for which, w_dram, dst, scale in ((0, wq, qth, SCALE), (1, wk, kth, 1.0)):
                    w_t = w_pool.tile([128, KCH, C], F32R, tag="w", name=f"w{which}")
                    for k in range(KCH):
                        weng = nc.sync if k % 2 == 0 else nc.scalar
                        weng.dma_start(w_t[:, k, :], w_dram[k * 128:(k + 1) * 128, :])
                    for nb in range(2):
                        pst = [
                            (ps_a if h < 4 else ps_b).tile(
                                [128, 512], F32, tag="psa" if h < 4 else "psb",
                                name=f"qk{which}_{nb}_{h}")
                            for h in range(H)
                        ]
                        for k in range(KCH):
                            if which == 0:
                                rhs_t = x_pool.tile([128, 512], F32R, tag="x")
                                xeng = nc.scalar if k % 2 == 0 else nc.sync
                                xeng.dma_start(
                                    rhs_t[:], xT[k * 128:(k + 1) * 128, nb * 512:(nb + 1) * 512]
                                )
                                rhs = rhs_t[:]
                            else:
                                rhs = yt_sb[:, k, nb * 512:(nb + 1) * 512]
                            for h in range(H):
                                nc.tensor.matmul(
                                    pst[h][:HD, :],
                                    w_t[:, k, h * HD:(h + 1) * HD],
                                    rhs,
                                    start=(k == 0),
                                    stop=(k == KCH - 1),
                                )
                        for h in range(H):
                            if which == 0:
                                nc.scalar.mul(dst[:, h, nb * 512:(nb + 1) * 512], pst[h][:HD, :], scale)
                            else:
                                nc.vector.tensor_copy(dst[:, h, nb * 512:(nb + 1) * 512], pst[h][:HD, :])

                # V (natural [m, c] layout scattered into vaug head slots)
                wv_t = w_pool.tile([128, KCH, C], F32R, tag="w", name="wv_t")
                for k in range(KCH):
                    weng = nc.sync if k % 2 == 0 else nc.scalar
                    weng.dma_start(wv_t[:, k, :], wv[k * 128:(k + 1) * 128, :])
                for c0, cw in ((0, 512), (512, 256)):
                    psv = [
                        (ps_a if mc < 4 else ps_b).tile(
                            [128, 512], F32, tag="psa" if mc < 4 else "psb",
                            name=f"v{c0}_{mc}")
                        for mc in range(NCH)
                    ]
                    for k in range(KCH):
                        for mc in range(NCH):
                            nc.tensor.matmul(
                                psv[mc][:, :cw],
                                yt_sb[:, k, mc * 128:(mc + 1) * 128],
                                wv_t[:, k, c0:c0 + cw],
                                start=(k == 0),
                                stop=(k == KCH - 1),
                            )
                    for mc in range(NCH):
                        c = c0
                        i = 0
                        while c < c0 + cw:
                            h = c // HD
                            hi = min((h + 1) * HD, c0 + cw)
                            dst_ap = vaug[:, mc, h, c - h * HD:hi - h * HD]
                            src_ap = psv[mc][:, c - c0:hi - c0]
                            if i % 2 == 0:
                                nc.vector.tensor_copy(dst_ap, src_ap)
                            else:
                                nc.scalar.copy(dst_ap, src_ap)
                            c = hi
                            i += 1

            # ---------------- attention ----------------
            with ExitStack() as att_ctx:
                rel_pool = att_ctx.enter_context(tc.tile_pool(name="rel", bufs=4))
                es_pool = att_ctx.enter_context(tc.tile_pool(name="es", bufs=4))
                bc_pool = att_ctx.enter_context(tc.tile_pool(name="bc", bufs=3))
                sm_pool = att_ctx.enter_context(tc.tile_pool(name="sm", bufs=2))

                for h in range(H):
                    oa0 = ps_b.tile([128, 512], F32, tag="psb", name=f"oa0_{h}")
                    oa1 = ps_b.tile([128, 512], F32, tag="psb", name=f"oa1_{h}")
                    for mc in range(NCH):
                        st0 = ps_a.tile([128, 512], F32, tag="psa", name="st0")
                        st1 = ps_a.tile([128, 512], F32, tag="psa", name="st1")
                        kt_sl = kth[:, h, mc * 128:(mc + 1) * 128]
                        rel_t = rel_pool.tile([128, N], F16, tag="rel")
                        (nc.sync if mc % 2 == 0 else nc.gpsimd).dma_start(rel_t[:], rel[h, mc * 128:(mc + 1) * 128, :])
                        nc.tensor.matmul(st0[:], kt_sl, qth[:, h, 0:512], start=True, stop=False)
                        nc.tensor.matmul(st1[:], kt_sl, qth[:, h, 512:1024], start=True, stop=False)
                        nc.tensor.matmul(st0[:], id_sb[:], rel_t[:, 0:512], start=False, stop=True)
                        nc.tensor.matmul(st1[:], id_sb[:], rel_t[:, 512:1024], start=False, stop=True)
                        es = es_pool.tile([128, N], F32R, tag="es")
                        nc.scalar.activation(es[:, 0:512], st0[:], EXP)
                        nc.scalar.activation(es[:, 512:1024], st1[:], EXP)
                        va = vaug[:, mc, h, :]
                        nc.tensor.matmul(oa0[:HD + 1, :], va, es[:, 0:512],
                                         start=(mc == 0), stop=(mc == NCH - 1))
                        nc.tensor.matmul(oa1[:HD + 1, :], va, es[:, 512:1024],
                                         start=(mc == 0), stop=(mc == NCH - 1))
                    # normalize: row HD of oa* holds the softmax denominators per n
                    sums = sm_pool.tile([128, N], F32, tag="sm")
                    nc.vector.tensor_copy(sums[HD:HD + 1, 0:512], oa0[HD:HD + 1, :])
                    nc.vector.tensor_copy(sums[HD:HD + 1, 512:1024], oa1[HD:HD + 1, :])
                    # partition shift 96 -> 0 via DMA (on HW partition_broadcast
                    # reads physical partition 0 regardless of the AP base)
                    nc.scalar.dma_start(sums[0:1, :], sums[HD:HD + 1, :])
                    bcb = bc_pool.tile([HD, N], F32, tag="bc")
                    nc.gpsimd.partition_broadcast(bcb[:], sums[0:1, :], channels=HD)
                    # 1/x via DVE Newton-Raphson custom op (~2 ULP)
                    scr = bc_pool.tile([HD, N], F32, tag="bc")
                    rcp = bc_pool.tile([HD, N], F32, tag="bc")
                    nc.vector.reciprocal_approx_accurate(rcp[:, 0:512], bcb[:, 0:512], scr[:, 0:512])
                    nc.vector.reciprocal_approx_accurate(rcp[:, 512:1024], bcb[:, 512:1024], scr[:, 512:1024])
                    nc.vector.tensor_tensor(at_hm[:, h, 0:512], oa0[:HD, :], rcp[:, 0:512], MUL)
                    nc.vector.tensor_tensor(at_hm[:, h, 512:1024], oa1[:HD, :], rcp[:, 512:1024], MUL)

            # ---------------- output projection ----------------
            with ExitStack() as proj_ctx:
                wp_pool = proj_ctx.enter_context(tc.tile_pool(name="wpp", bufs=1))
                ob_pool = proj_ctx.enter_context(tc.tile_pool(name="ob", bufs=3))
                wp_t = wp_pool.tile([HD, H, C], F32R, tag="wpp")
                for hh in range(H):
                    (nc.sync if hh % 2 == 0 else nc.gpsimd).dma_start(wp_t[:, hh, :], wp[:, hh, :])
                for c0, cw in ((0, 512), (512, 256)):
                    po = [
                        (ps_a if j < 4 else ps_b).tile(
                            [128, 512], F32, tag="psa" if j < 4 else "psb",
                            name=f"po{c0}_{j}")
                        for j in range(NCH)
                    ]
                    for h in range(H):
                        for j in range(NCH):
                            nc.tensor.matmul(
                                po[j][:, :cw],
                                at_hm[:, h, j * 128:(j + 1) * 128],
                                wp_t[:, h, c0:c0 + cw],
                                start=(h == 0),
                                stop=False,
                            )
                    for j in range(NCH):
                        # + bias via K=1 ones matmul (broadcast bp over partitions)
                        nc.tensor.matmul(po[j][:, :cw], ones[0:1, :], bp_sb[0:1, c0:c0 + cw],
                                         start=False, stop=True)
                        ot = ob_pool.tile([128, 512], F32, tag="ob")
                        nc.vector.tensor_copy(ot[:, :cw], po[j][:, :cw])
                        nc.scalar.dma_start(out[j * 128:(j + 1) * 128, c0:c0 + cw], ot[:, :cw])

    nc.compile()
    _CACHE["nc"] = nc
    return nc


def make_in_maps(x, y, relative_pos, Wq, Wk, Wv, Wp, bp):
    x = np.asarray(x, dtype=np.float32)
    y = np.asarray(y, dtype=np.float32)
    relative_pos = np.asarray(relative_pos, dtype=np.float32)
    Wq = np.asarray(Wq, dtype=np.float32)
    Wk = np.asarray(Wk, dtype=np.float32)
    Wv = np.asarray(Wv, dtype=np.float32)
    Wp = np.asarray(Wp, dtype=np.float32)
    bp = np.asarray(bp, dtype=np.float32)

    wqT = np.ascontiguousarray(Wq.T)
    wkT = np.ascontiguousarray(Wk.T)
    wvT = np.ascontiguousarray(Wv.T)
    # Wp.T is [c'=h*HD+d, c]; head-major: [d, h, c]
    wp_hm = np.ascontiguousarray(Wp.T.reshape(H, HD, C).transpose(1, 0, 2))
    relT = np.ascontiguousarray(relative_pos.transpose(0, 2, 1)).astype(np.float16)
    bp2 = np.ascontiguousarray(bp.reshape(1, C))

    in_maps = []
    for b in range(B):
        in_maps.append({
            "xT": np.ascontiguousarray(x[b].T),
            "yT": np.ascontiguousarray(y[b].T),
            "wq": wqT, "wk": wkT, "wv": wvT, "wp": wp_hm, "bp": bp2,
            "rel": relT,
            "onesr": np.ones((1, 128), dtype=np.float32),
            "ident": np.eye(128, dtype=np.float16),
            "onesv": np.ones((128, NCH * H), dtype=np.float32),
        })
    return in_maps


def kernel(x, y, relative_pos, H=None, W=None, Wq=None, Wk=None, Wv=None, Wp=None, bp=None,
           **extra):
    nc = build_bass()
    in_maps = make_in_maps(x, y, relative_pos, Wq, Wk, Wv, Wp, bp)
    res = run_bass_kernel_spmd(nc, in_maps, list(range(B)))
    return np.stack([res.results[b]["out"] for b in range(B)], axis=0)
